# revision 1
# baseline (speedup 1.0000x reference)
"""CapsuleLayer dynamic-routing kernel for 8 Trainium2 NeuronCores.

Problem: x [128, 2048, 8], W [32, 2048, 16, 8] (fp32)
  u_hat[b,j,i,d] = sum_p W[j,i,d,p] * x[b,i,p]
  3 rounds of routing-by-agreement (softmax over j, squash), no
  persistent logits needed: b_k = (sum_{m<k} out_m) . u_hat, so each
  round is a streaming pass over i needing only O_k = sum out_m.

Sharding: i (input capsules) split 8 ways; every core holds the full
batch B=128 on SBUF partitions. Per-round partial sums s[b,(j,d)]
(256KB/core) are reduced on the host between the three launches.
"""

import numpy as np
from contextlib import ExitStack

import concourse.bass as bass
import concourse.mybir as mybir
from concourse import tile
from concourse.bass_utils import run_bass_kernel_spmd

# ---------------------------------------------------------------------------
# Shapes (hardcoded for this problem)
B, I, P = 128, 2048, 8
J, D = 32, 16
JD = J * D               # 512
N_CORES = 8
I_LOC = I // N_CORES     # 256
EPS = 1e-7
GROUP = 4                # i's per routing group (psum tile = GROUP banks)
N_GROUPS = I_LOC // GROUP

_f32 = mybir.dt.float32


# ---------------------------------------------------------------------------
# Walrus compat: this toolchain rejects sync waits on InstDrain and >2 on
# InstEventSemaphore. Emit the waits as standalone nops before the drain.
def _apply_tile_compat():
    from concourse.vector_clock import ScopedClock

    def _strip_waits(inst):
        si = inst.sync_info
        if not si or not si.on_wait:
            return []
        waits = list(si.on_wait)
        si.on_wait = []
        inst.sync_info = si
        return waits

    def _nop_with_wait(eng, w):
        nop = eng.nop(nofuse=True, hint="drain_wait_split")
        nsi = nop.ins.sync_info
        if nsi is None:
            nsi = mybir.SyncInfo(on_wait=[], on_update=[])
        nsi.on_wait = list(nsi.on_wait or []) + [w]
        nop.ins.sync_info = nsi

    def _patched_multi_engine_barrier(self, engines):
        for inst in bass._bass_rust._multi_engine_barrier_insts(
            self, list(engines)
        ):
            eng = self.engines[inst.engine]
            for w in _strip_waits(inst):
                _nop_with_wait(eng, w)
            eng.add_instruction(inst)

    def _patched_drain_and_barrier(self, tick_clock, wait_clock):
        nop_inst = self.nc.sync.nop(nofuse=True, hint="drain_wait_split")
        wait_clock.add_sem_waits(
            nop_inst.ins, ScopedClock({None: tick_clock.global_clock})
        )
        si = nop_inst.ins.sync_info
        if si and si.on_wait and len(si.on_wait) > 1:
            extra = list(si.on_wait[1:])
            si.on_wait = [si.on_wait[0]]
            nop_inst.ins.sync_info = si
            for w in extra:
                _nop_with_wait(self.nc.sync, w)
        self.nc.sync.drain()

        self.nc.all_engine_barrier()
        assert self.sems is not None
        popped = self.nc._tile_sem_poison_stack.pop()
        assert popped is self._sem_poison
        self.nc.clear_and_free_semaphores(list(self.sems.allocated().values()))
        # No trailing all_engine_barrier: every engine is already past the
        # pre-clear barrier (done touching semaphores), nothing reads them
        # afterwards, and NEFF completion only needs each engine to halt.

    # Scheduled body instructions can also end up with >1 wait (e.g. a
    # matmul waiting on two DMAs). Spill extras onto same-engine NoOps
    # inserted immediately before the instruction.
    _WAIT_CAPS = {"InstDrain": 0, "InstEventSemaphore": 2}
    _orig_add_instruction = tile.TileContext._add_instruction

    def _patched_add_instruction(self, inst):
        si = inst.sync_info
        cap = _WAIT_CAPS.get(type(inst).__name__, 1)
        if si and si.on_wait and len(si.on_wait) > cap:
            waits = list(si.on_wait)
            si.on_wait = waits[:cap]
            inst.sync_info = si
            for w in waits[cap:]:
                nop = mybir.InstNoOp(
                    name=f"I-{self.nc.next_id()}-waitspill", ins=[], outs=[]
                )
                nop.engine = inst.engine
                nop.sync_info = mybir.SyncInfo(on_wait=[w], on_update=[])
                _orig_add_instruction(self, nop)
        _orig_add_instruction(self, inst)

    bass.Bass.multi_engine_barrier = _patched_multi_engine_barrier
    tile.TileContext._drain_and_barrier = _patched_drain_and_barrier
    tile.TileContext._add_instruction = _patched_add_instruction


_apply_tile_compat()


# ---------------------------------------------------------------------------
# Launch 1: s0_part[b,(j,d)] = sum_{i local} u_hat[b,j,i,d]
# (iteration 0 has exactly uniform c = 1/32, applied on the host)
def build_l1():
    nc = bass.Bass("TRN2", target_bir_lowering=False, debug=False)
    n_chunks = (I_LOC * P) // 128  # 16
    xw1 = nc.dram_tensor(
        "xw1", [n_chunks, 128, B + JD], _f32, kind="ExternalInput").ap()
    sp = nc.dram_tensor("sp", [B, JD], _f32, kind="ExternalOutput").ap()
    with ExitStack() as ctx:
        tc = ctx.enter_context(tile.TileContext(nc))
        xpool = ctx.enter_context(tc.tile_pool(name="xw1", bufs=4))
        ppool = ctx.enter_context(tc.tile_pool(name="ps", bufs=1, space="PSUM"))
        opool = ctx.enter_context(tc.tile_pool(name="o", bufs=1))
        psum = ppool.tile([B, JD], _f32)
        for q in range(n_chunks):
            t = xpool.tile([128, B + JD], _f32)
            nc.sync.dma_start(t[:], xw1[q])
            nc.tensor.matmul(
                psum[:], lhsT=t[:, :B], rhs=t[:, B:],
                start=(q == 0), stop=(q == n_chunks - 1),
            )
        out = opool.tile([B, JD], _f32)
        nc.scalar.copy(out[:], psum[:])
        nc.sync.dma_start(sp[:], out[:])
    return nc


# ---------------------------------------------------------------------------
# Launches 2 & 3: one routing round.
#   g[b,j,i]  = sum_d O[b,j,d] * u_hat[b,j,i,d]
#   c         = softmax_j(g)
#   s_part    = sum_{i local} c * u_hat
#
# x and W arrive interleaved in 16-i blocks ("xw": per block, the x
# slab [P, 16*B] then the W slab [P, 16*JD], both p-major) so each
# block is one large DMA instead of 16 small ones.
BLK = 16                       # i's per DMA block
N_BLKS = I_LOC // BLK          # 16
XW_X = BLK * B                 # 2048 x columns per block
XW_W = BLK * JD                # 8192 W columns per block
XW_COLS = XW_X + XW_W          # 10240


def build_l2():
    nc = bass.Bass("TRN2", target_bir_lowering=False, debug=False)
    xw = nc.dram_tensor(
        "xw", [N_BLKS, P, XW_COLS], _f32, kind="ExternalInput").ap()
    x2d = nc.dram_tensor("x2", [B, I_LOC * P], _f32, kind="ExternalInput").ap()
    w2d = nc.dram_tensor(
        "w2", [J, D, I_LOC * P], _f32, kind="ExternalInput").ap()
    otd = nc.dram_tensor("ot", [D, J * B], _f32, kind="ExternalInput").ap()
    sp = nc.dram_tensor("sp", [B, JD], _f32, kind="ExternalOutput").ap()

    IP = I_LOC * P  # 2048

    with ExitStack() as ctx:
        tc = ctx.enter_context(tile.TileContext(nc))
        wpool = ctx.enter_context(tc.tile_pool(name="xw", bufs=2))
        tpool = ctx.enter_context(tc.tile_pool(name="tmp", bufs=2))
        gpool = ctx.enter_context(tc.tile_pool(name="g", bufs=2))
        bpool = ctx.enter_context(tc.tile_pool(name="big", bufs=1))
        apool = ctx.enter_context(tc.tile_pool(name="acc", bufs=1))

        # wide accumulators: one GROUP-lane per i-position, reduced once
        # at the end. Two of them so even groups accumulate on DVE and
        # odd groups on GpSimd, halving the DVE add chain.
        s_wide = apool.tile([B, GROUP * JD], _f32)
        nc.gpsimd.memset(s_wide[:], 0.0)
        s_wide2 = apool.tile([B, GROUP * JD], _f32)
        nc.gpsimd.memset(s_wide2[:], 0.0)

        # ---- phase A: g[b,(j,i)] = sum_p x2[b,(i,p)] * (O_j @ W2_j)[b,(i,p)]
        # g_all is reused in place for e = exp(g - m) and then c (softmax
        # numerator / weights): every op is elementwise with identical
        # input/output traversal order.
        g_all = bpool.tile([B, J * I_LOC], _f32)

        with tc.tile_pool(name="vps", bufs=2, space="PSUM") as vppool, \
             tc.tile_pool(name="pa", bufs=1) as papool, \
             tc.tile_pool(name="w2", bufs=2) as w2pool:
            x2 = papool.tile([B, IP], _f32)
            nc.sync.dma_start(x2[:], x2d[:])
            ot = papool.tile([D, J * B], _f32)
            nc.sync.dma_start(ot[:], otd[:])
            for j in range(J):
                w2t = w2pool.tile([D, IP], _f32)
                nc.sync.dma_start(w2t[:], w2d[j])
                vps = vppool.tile([B, IP], _f32)
                for q in range(IP // 512):
                    nc.tensor.matmul(
                        vps[:, q * 512:(q + 1) * 512],
                        lhsT=ot[:, j * B:(j + 1) * B],
                        rhs=w2t[:, q * 512:(q + 1) * 512],
                        start=True, stop=True,
                    )
                xv = tpool.tile([B, IP], _f32)
                nc.vector.tensor_tensor(
                    xv[:], x2[:], vps[:], op=mybir.AluOpType.mult,
                )
                nc.vector.reduce_sum(
                    g_all[:, j * I_LOC:(j + 1) * I_LOC],
                    xv[:].rearrange("b (i p) -> b i p", i=I_LOC, p=P),
                    axis=mybir.AxisListType.X,
                )

        # ---- softmax over j (free-dim strided, one shot for all i).
        # No max-subtraction: g = O.u_hat with squashed O (|O_j| < 1) is
        # bounded well inside exp's fp32 range, and softmax is shift-
        # invariant, so exp(g)/sum exp(g) matches the reference exactly.
        gjv = g_all[:].rearrange("b (j i) -> b j i", j=J, i=I_LOC)
        giv = g_all[:].rearrange("b (j i) -> b i j", j=J, i=I_LOC)
        nc.scalar.activation(
            g_all[:], g_all[:], mybir.ActivationFunctionType.Exp
        )
        Z = bpool.tile([B, I_LOC], _f32)
        nc.vector.reduce_sum(Z[:], giv, axis=mybir.AxisListType.X)
        Zr = bpool.tile([B, I_LOC], _f32)
        nc.vector.reciprocal(Zr[:], Z[:])
        nc.vector.tensor_tensor(
            gjv, gjv, Zr[:].unsqueeze(1).broadcast_to([B, J, I_LOC]),
            op=mybir.AluOpType.mult,
        )
        c_v = giv

        # ---- phase B: s += sum_i c * u_hat, u_hat recomputed per group.
        # The weighted tiles w are accumulated on the PE into a resident
        # PSUM region via identity matmuls (start=False), so the DVE only
        # does the c-multiply.
        ppool = ctx.enter_context(tc.tile_pool(name="ps", bufs=2, space="PSUM"))
        xw_tiles = {}
        for gi in range(N_GROUPS):
            blk, sub = divmod(gi * GROUP, BLK)
            if sub == 0:
                xwt = wpool.tile([P, XW_COLS], _f32)
                nc.sync.dma_start(xwt[:], xw[blk])
                xw_tiles[blk] = xwt
            xwt = xw_tiles[blk]
            psum = ppool.tile([B, GROUP * JD], _f32)
            for t in range(GROUP):
                ib = sub + t           # i index within the block
                nc.tensor.matmul(
                    psum[:, t * JD:(t + 1) * JD],
                    lhsT=xwt[:, ib * B:(ib + 1) * B],
                    rhs=xwt[:, XW_X + ib * JD:XW_X + (ib + 1) * JD],
                    start=True, stop=True,
                )
            pv = psum[:].rearrange("b (i j d) -> b i j d", i=GROUP, j=J, d=D)
            cslice = c_v[:, gi * GROUP:(gi + 1) * GROUP, :]
            w = tpool.tile([B, GROUP * JD], _f32)
            wv = w[:].rearrange("b (i j d) -> b i j d", i=GROUP, j=J, d=D)
            nc.vector.tensor_tensor(
                wv, pv, cslice.unsqueeze(3).broadcast_to([B, GROUP, J, D]),
                op=mybir.AluOpType.mult,
            )
            if gi % 2 == 0:
                nc.vector.tensor_add(s_wide[:], s_wide[:], w[:])
            else:
                nc.gpsimd.tensor_add(s_wide2[:], s_wide2[:], w[:])

        nc.vector.tensor_add(s_wide[:], s_wide[:], s_wide2[:])
        s_acc = gpool.tile([B, JD], _f32)
        nc.vector.reduce_sum(
            s_acc[:],
            s_wide[:].rearrange("b (i jd) -> b jd i", i=GROUP, jd=JD),
            axis=mybir.AxisListType.X,
        )
        nc.sync.dma_start(sp[:], s_acc[:])
    return nc


# ---------------------------------------------------------------------------
# Host glue
def _squash(s):
    v = s.reshape(B, J, D)
    s2 = np.sum(np.square(v), axis=-1, keepdims=True)
    scale = s2 / (1.0 + s2) / np.sqrt(s2 + EPS)
    return (scale * v).astype(np.float32)


_cache = {}


def _get_nc(name):
    if name not in _cache:
        _cache[name] = build_l1() if name == "l1" else build_l2()
    return _cache[name]


def _prep_inputs(x, W):
    """Per-core host-side re-layouts (all fp32, cheap transposes)."""
    per_core = []
    for c in range(N_CORES):
        sl = slice(c * I_LOC, (c + 1) * I_LOC)
        xc = x[:, sl, :]                                   # [B, I_LOC, P]
        wc = W[:, sl, :, :]                                # [J, I_LOC, D, P]
        xp = np.ascontiguousarray(
            xc.transpose(1, 2, 0).reshape(I_LOC * P, B))   # (i,p),b
        wt = np.ascontiguousarray(
            wc.transpose(1, 3, 0, 2).reshape(I_LOC * P, JD))  # (i,p),(j,d)
        # L1: interleave x/W per 128-row chunk so each chunk is one DMA
        n_chunks = (I_LOC * P) // 128
        xw1 = np.empty((n_chunks, 128, B + JD), np.float32)
        xw1[:, :, :B] = xp.reshape(n_chunks, 128, B)
        xw1[:, :, B:] = wt.reshape(n_chunks, 128, JD)
        # interleaved blocks for L2/L3: per 16-i block, [P, 16*B | 16*JD]
        xb = xc.transpose(2, 1, 0).reshape(P, N_BLKS, BLK * B)  # p,(blk,i*b)
        wb = wc.transpose(3, 1, 0, 2).reshape(P, N_BLKS, BLK, JD)
        xw = np.empty((N_BLKS, P, XW_COLS), np.float32)
        xw[:, :, :XW_X] = xb.transpose(1, 0, 2)
        xw[:, :, XW_X:] = wb.transpose(1, 0, 2, 3).reshape(N_BLKS, P, XW_W)
        # V-trick layouts
        x2 = np.ascontiguousarray(xc.reshape(B, I_LOC * P))      # b,(i,p)
        w2 = np.ascontiguousarray(
            wc.transpose(0, 2, 1, 3).reshape(J, D, I_LOC * P))   # j,d,(i,p)
        per_core.append({"xw1": xw1, "xw": xw, "x2": x2, "w2": w2})
    return per_core


def _ot_layout(O):
    """O [B, JD] -> lhsT layout [D, J*B] for the V matmuls."""
    return np.ascontiguousarray(
        O.reshape(B, J, D).transpose(2, 1, 0).reshape(D, J * B))


def _run(nc, in_maps, **kw):
    res = run_bass_kernel_spmd(nc, in_maps, list(range(N_CORES)), **kw)
    return res


def kernel(x, W, _collect_times=None):
    x = np.asarray(x, dtype=np.float32)
    W = np.asarray(W, dtype=np.float32)
    pc = _prep_inputs(x, W)

    nc1 = _get_nc("l1")
    nc2 = _get_nc("l2")

    r1 = _run(nc1, [{"xw1": p["xw1"]} for p in pc])
    s0 = np.sum([r1.results[c]["sp"] for c in range(N_CORES)], axis=0)
    s0 *= (1.0 / J)
    out0 = _squash(s0)
    O1 = out0.reshape(B, JD)

    ot1 = _ot_layout(O1)
    r2 = _run(nc2, [
        {"xw": p["xw"], "x2": p["x2"], "w2": p["w2"], "ot": ot1}
        for p in pc
    ])
    s1 = np.sum([r2.results[c]["sp"] for c in range(N_CORES)], axis=0)
    out1 = _squash(s1)
    O2 = (out0 + out1).reshape(B, JD)

    ot2 = _ot_layout(O2)
    r3 = _run(nc2, [
        {"xw": p["xw"], "x2": p["x2"], "w2": p["w2"], "ot": ot2}
        for p in pc
    ])
    s2 = np.sum([r3.results[c]["sp"] for c in range(N_CORES)], axis=0)
    out2 = _squash(s2)

    if _collect_times is not None:
        for r in (r1, r2, r3):
            _collect_times.append(r.exec_time_ns)
    return out2



# revision 15
# speedup vs baseline: 3.0748x; 3.0748x over previous
"""CapsuleLayer dynamic-routing kernel for 8 Trainium2 NeuronCores.

Problem: x [128, 2048, 8], W [32, 2048, 16, 8] (fp32)
  u_hat[b,j,i,d] = sum_p W[j,i,d,p] * x[b,i,p]
  3 rounds of routing-by-agreement (softmax over j, squash).
  b_k = (sum_{m<k} out_m) . u_hat, so each round is a streaming pass
  over i needing only O_cum = sum of previous outputs.

Sharding: i (input capsules) split 8 ways; every core holds the full
batch B=128. Per-round partial sums s[b,(j,d)] are reduced on the host
between the three launches (squash also on host, it's tiny).

Per-round dataflow on each core (transposed [i, (p,b)] layout, bf16):
  phase 1 (per j, per i-chunk h of 128):
    A[(io,p), b]  = sum_d W[j,i,d,p] O[b,j,d]      (PE, contract d=16)
    eA            = A (PSUM f32 -> SBUF bf16)      (Act copy)
    m             = eA * x                         (DVE bf16 2x)
    g[i, b]       = sum_p m                        (PE selector matmul)
    e             = exp(g)                         (Act, PSUM->SBUF bf16)
  softmax norm (per i-chunk): Z = sum_j e (Pool), Zr = 1/Z (DVE),
    xh = x * Zr (folded once, so c = e needs no extra pass)
  phase 2 (per j, per i-chunk):
    cx[i,(p,b)]   = e_j * xh                       (DVE/Pool bf16)
    s[b,(j,d)]   += cx^T W                         (PE, contract (i,p))
"""

import numpy as np
import ml_dtypes
from contextlib import ExitStack

import concourse.bass as bass
import concourse.mybir as mybir
from concourse import tile
from concourse.bass_utils import run_bass_kernel_spmd

# ---------------------------------------------------------------------------
# Shapes (hardcoded for this problem)
B, I, P = 128, 2048, 8
J, D = 32, 16
JD = J * D               # 512
N_CORES = 8
I_LOC = I // N_CORES     # 256
EPS = 1e-7

_f32 = mybir.dt.float32
_bf16 = mybir.dt.bfloat16
_bf = ml_dtypes.bfloat16

N_H = 2                  # i-chunks of 128 per core
JH = J // 8              # 4  (j // 8 index in w2t/otd layouts)

# Per-section engine assignment patterns (index j % len):
#   mult paths: 'a' = Act evac + DVE bf16 mult, 'd' = DVE direct from PSUM,
#               'p' = Pool direct from PSUM
#   cx paths:   'd' = DVE, 'p' = Pool
GSZ = 4                  # j's per G/exp group
MIX_A = "aadaadaa"       # section 1 mults (P1 h0 alone)
MIX_B = "aadaadaa"       # section 2 mults (P1 h1 + P2 h0)
CX_B = "dddpd"           # section 2 cx
CX_C = "dddpdp"          # tail cx


# ---------------------------------------------------------------------------
# Walrus compat: this toolchain rejects sync waits on InstDrain and >2 on
# InstEventSemaphore. Emit the waits as standalone nops before the drain.
def _apply_tile_compat():
    from concourse.vector_clock import ScopedClock

    def _strip_waits(inst):
        si = inst.sync_info
        if not si or not si.on_wait:
            return []
        waits = list(si.on_wait)
        si.on_wait = []
        inst.sync_info = si
        return waits

    def _nop_with_wait(eng, w):
        nop = eng.nop(nofuse=True, hint="drain_wait_split")
        nsi = nop.ins.sync_info
        if nsi is None:
            nsi = mybir.SyncInfo(on_wait=[], on_update=[])
        nsi.on_wait = list(nsi.on_wait or []) + [w]
        nop.ins.sync_info = nsi

    def _patched_multi_engine_barrier(self, engines):
        for inst in bass._bass_rust._multi_engine_barrier_insts(
            self, list(engines)
        ):
            eng = self.engines[inst.engine]
            for w in _strip_waits(inst):
                _nop_with_wait(eng, w)
            eng.add_instruction(inst)

    def _patched_drain_and_barrier(self, tick_clock, wait_clock):
        nop_inst = self.nc.sync.nop(nofuse=True, hint="drain_wait_split")
        wait_clock.add_sem_waits(
            nop_inst.ins, ScopedClock({None: tick_clock.global_clock})
        )
        si = nop_inst.ins.sync_info
        if si and si.on_wait and len(si.on_wait) > 1:
            extra = list(si.on_wait[1:])
            si.on_wait = [si.on_wait[0]]
            nop_inst.ins.sync_info = si
            for w in extra:
                _nop_with_wait(self.nc.sync, w)
        self.nc.sync.drain()

        self.nc.all_engine_barrier()
        assert self.sems is not None
        popped = self.nc._tile_sem_poison_stack.pop()
        assert popped is self._sem_poison
        self.nc.clear_and_free_semaphores(list(self.sems.allocated().values()))
        # No trailing all_engine_barrier: every engine is already past the
        # pre-clear barrier (done touching semaphores), nothing reads them
        # afterwards, and NEFF completion only needs each engine to halt.

    # Scheduled body instructions can also end up with >1 wait (e.g. a
    # matmul waiting on two DMAs). Spill extras onto same-engine NoOps
    # inserted immediately before the instruction.
    _WAIT_CAPS = {"InstDrain": 0, "InstEventSemaphore": 2}
    _orig_add_instruction = tile.TileContext._add_instruction

    def _patched_add_instruction(self, inst):
        si = inst.sync_info
        cap = _WAIT_CAPS.get(type(inst).__name__, 1)
        if si and si.on_wait and len(si.on_wait) > cap:
            waits = list(si.on_wait)
            si.on_wait = waits[:cap]
            inst.sync_info = si
            for w in waits[cap:]:
                nop = mybir.InstNoOp(
                    name=f"I-{self.nc.next_id()}-waitspill", ins=[], outs=[]
                )
                nop.engine = inst.engine
                nop.sync_info = mybir.SyncInfo(on_wait=[w], on_update=[])
                _orig_add_instruction(self, nop)
        _orig_add_instruction(self, inst)

    bass.Bass.multi_engine_barrier = _patched_multi_engine_barrier
    tile.TileContext._drain_and_barrier = _patched_drain_and_barrier
    tile.TileContext._add_instruction = _patched_add_instruction


_apply_tile_compat()


# ---------------------------------------------------------------------------
# Launch 1 (round 0): s0_part[b,(j,d)] = sum_{i local} u_hat[b,j,i,d]
# (iteration 0 has exactly uniform c = 1/32, applied on the host)
def build_l1():
    nc = bass.Bass("TRN2", target_bir_lowering=False, debug=False)
    xi = nc.dram_tensor("xi", [N_H, 128, P * B], _bf16, kind="ExternalInput").ap()
    w4 = nc.dram_tensor("w4", [N_H, 128, P * JD], _bf16, kind="ExternalInput").ap()
    sp = nc.dram_tensor("sp", [B, JD], _f32, kind="ExternalOutput").ap()
    with ExitStack() as ctx:
        tc = ctx.enter_context(tile.TileContext(nc))
        xpool = ctx.enter_context(tc.tile_pool(name="x", bufs=1))
        wpool = ctx.enter_context(tc.tile_pool(name="w", bufs=2))
        ppool = ctx.enter_context(tc.tile_pool(name="ps", bufs=1, space="PSUM"))
        opool = ctx.enter_context(tc.tile_pool(name="o", bufs=1))
        xs = []
        for h in range(N_H):
            xt = xpool.tile([128, P * B], _bf16)
            nc.sync.dma_start(xt[:], xi[h])
            xs.append(xt)
        psum = ppool.tile([B, JD], _f32)
        for h in range(N_H):
            wt = wpool.tile([128, P * JD], _bf16)
            nc.sync.dma_start(wt[:], w4[h])
            for p in range(P):
                nc.tensor.matmul(
                    psum[:],
                    lhsT=xs[h][:, p * B:(p + 1) * B],
                    rhs=wt[:, p * JD:(p + 1) * JD],
                    start=(h == 0 and p == 0),
                    stop=(h == N_H - 1 and p == P - 1),
                )
        out = opool.tile([B, JD], _f32)
        nc.scalar.copy(out[:], psum[:])
        nc.sync.dma_start(sp[:], out[:])
    return nc


# ---------------------------------------------------------------------------
# Launches 2 & 3: one routing round (see module docstring).
def build_l2():
    nc = bass.Bass("TRN2", target_bir_lowering=False, debug=False)
    # Stage-1 contracts over the full 128 partitions (jm, d); the otd rhs
    # is zero outside the target j's 16 d-rows, so base partitions stay 0.
    # w2t[jh][jm*16+d, c*128 + io*8+p] = W[8jh+jm, 16c+io, d, p]
    w2t = nc.dram_tensor("w2t", [JH, 128, 16 * 128], _bf16,
                         kind="ExternalInput").ap()
    w4 = nc.dram_tensor("w4", [N_H, 128, P * JD], _bf16, kind="ExternalInput").ap()
    xti = nc.dram_tensor("xti", [128, 16 * B], _bf16, kind="ExternalInput").ap()
    xi = nc.dram_tensor("xi", [N_H, 128, P * B], _bf16, kind="ExternalInput").ap()
    # otd[jh][jm*16+d, jm2*B + b] = O[b, 8jh+jm2, d] if jm==jm2 else 0
    otd = nc.dram_tensor("otd", [JH, 128, 8 * B], _bf16,
                         kind="ExternalInput").ap()
    # sel[io*8+p, cc*128 + cc*16+io] = 1 (one selector block per cc)
    sel = nc.dram_tensor("sel", [128, 8 * 128], _bf16, kind="ExternalInput").ap()
    sp = nc.dram_tensor("sp", [B, JD], _f32, kind="ExternalOutput").ap()

    with ExitStack() as ctx:
        tc = ctx.enter_context(tile.TileContext(nc))
        inpool = ctx.enter_context(tc.tile_pool(name="in", bufs=1))
        apool = ctx.enter_context(tc.tile_pool(name="aps", bufs=2, space="PSUM"))
        gpool = ctx.enter_context(tc.tile_pool(name="gps", bufs=2, space="PSUM"))
        spool = ctx.enter_context(tc.tile_pool(name="sps", bufs=1, space="PSUM"))
        eapool = ctx.enter_context(tc.tile_pool(name="ea", bufs=3))
        mpool = ctx.enter_context(tc.tile_pool(name="m", bufs=4))
        cpool = ctx.enter_context(tc.tile_pool(name="cx", bufs=3))
        bigpool = ctx.enter_context(tc.tile_pool(name="big", bufs=1))
        zppool = ctx.enter_context(tc.tile_pool(name="zp", bufs=14))

        # ---- input loads (order matters for the pipeline head)
        w2t_t = []
        otd_t = []
        for jh in range(JH):
            t = inpool.tile([128, 16 * 128], _bf16, name=f"w2t_t{jh}")
            w2t_t.append(t)
            t2 = inpool.tile([128, 8 * B], _bf16, name=f"otd_t{jh}")
            otd_t.append(t2)
        nc.scalar.dma_start(otd_t[0][:], otd[0])
        nc.sync.dma_start(w2t_t[0][:], w2t[0])
        sel_t = inpool.tile([128, 8 * 128], _bf16)
        nc.scalar.dma_start(sel_t[:], sel[:])
        xti_t = inpool.tile([128, 16 * B], _bf16)
        nc.sync.dma_start(xti_t[:], xti[:])
        for jh in range(1, JH):
            nc.sync.dma_start(w2t_t[jh][:], w2t[jh])
            nc.scalar.dma_start(otd_t[jh][:], otd[jh])

        # PE warm-up: ramp the tensor engine to full clock while the first
        # DMAs land (p-state model needs ~3us of continuous activity).
        wup = inpool.tile([128, 128], _bf16)
        nc.vector.memset(wup[:], 0.0)
        wu_ps = spool.tile([B, 512], _f32, name="wu_ps")
        for _ in range(10):
            nc.tensor.matmul(wu_ps[:, :128], lhsT=wup[:], rhs=wup[:],
                             start=True, stop=True)
        xi_t = []
        for h in range(N_H):
            t = inpool.tile([128, P * B], _bf16, name=f"xi_t{h}")
            nc.scalar.dma_start(t[:], xi[h])
            xi_t.append(t)
        w4_t = []
        for h in range(N_H):
            t = inpool.tile([128, P * JD], _bf16, name=f"w4_t{h}")
            nc.scalar.dma_start(t[:], w4[h])
            w4_t.append(t)

        eT = [bigpool.tile([128, J * B], _bf16, name=f"eT{h}") for h in range(N_H)]
        zparts = {0: [], 1: []}
        xh = [bigpool.tile([128, P * B], _bf16, name=f"xh{h}") for h in range(N_H)]

        def phase1_j(h, j, path):
            """A = W.O ; m = A*x for one (h, j)."""
            jm, jh = j % 8, j // 8
            a_ps = apool.tile([128, 8 * B], _f32)
            for cc in range(8):
                c = h * 8 + cc
                nc.tensor.matmul(
                    a_ps[:, cc * B:(cc + 1) * B],
                    lhsT=w2t_t[jh][:, c * 128:(c + 1) * 128],
                    rhs=otd_t[jh][:, jm * B:(jm + 1) * B],
                    start=True, stop=True,
                )
            m = mpool.tile([128, 8 * B], _bf16)
            xsl = xti_t[:, h * 8 * B:(h + 1) * 8 * B]
            if path == "d":
                nc.vector.tensor_tensor(
                    m[:], a_ps[:], xsl, op=mybir.AluOpType.mult)
            else:
                ea = eapool.tile([128, 8 * B], _bf16)
                nc.scalar.copy(ea[:], a_ps[:])
                nc.vector.tensor_tensor(
                    m[:], ea[:], xsl, op=mybir.AluOpType.mult)
            return m

        def sel_reduce(g_ps, m, jj):
            # g[:, jj] = sum_cc SEL_cc^T m_cc  (accumulating, full M=128)
            for cc in range(8):
                nc.tensor.matmul(
                    g_ps[:, jj * B:(jj + 1) * B],
                    lhsT=sel_t[:, cc * 128:(cc + 1) * 128],
                    rhs=m[:, cc * B:(cc + 1) * B],
                    start=(cc == 0), stop=(cc == 7),
                )

        # Software-pipelined phase 1 over one i-chunk h: stage1/evac/mult
        # for j runs STAGGER j-slots ahead of the SEL reduce for j, so the
        # PE never blocks on the Act->DVE chain. interleave(j) lets the
        # caller splice phase-2 work of the other chunk into each slot.
        STAGGER = 2

        def phase1_chunk(h, mix, interleave=None):
            pending = []   # (m_tile, j)
            g_tiles = {}
            for j in range(J + STAGGER):
                if j < J:
                    m = phase1_j(h, j, mix[j % len(mix)])
                    pending.append((m, j))
                if j >= STAGGER:
                    m0, j0 = pending.pop(0)
                    if j0 % GSZ == 0:
                        g_tiles[j0 // GSZ] = gpool.tile(
                            [128, GSZ * B], _f32, name="g_ps")
                    sel_reduce(g_tiles[j0 // GSZ], m0, j0 % GSZ)
                    if j0 % GSZ == GSZ - 1:
                        jg = j0 // GSZ
                        esl = eT[h][:, jg * GSZ * B:(jg + 1) * GSZ * B]
                        nc.scalar.activation(
                            esl, g_tiles.pop(jg)[:],
                            mybir.ActivationFunctionType.Exp,
                        )

                if interleave is not None and j < J:
                    interleave(j)

        def znorm(h):
            """Z = sum_j e (bf16 add tree over j-blocks); xh = x * (1/Z)."""
            e = eT[h]
            t1 = bigpool.tile([128, 16 * B], _bf16, name=f"zt1_{h}")
            nc.vector.tensor_tensor(
                t1[:], e[:, :16 * B], e[:, 16 * B:], op=mybir.AluOpType.add)
            t2 = bigpool.tile([128, 8 * B], _bf16, name=f"zt2_{h}")
            nc.vector.tensor_tensor(
                t2[:], t1[:, :8 * B], t1[:, 8 * B:], op=mybir.AluOpType.add)
            t3 = bigpool.tile([128, 4 * B], _bf16, name=f"zt3_{h}")
            nc.vector.tensor_tensor(
                t3[:], t2[:, :4 * B], t2[:, 4 * B:], op=mybir.AluOpType.add)
            t4 = bigpool.tile([128, 2 * B], _bf16, name=f"zt4_{h}")
            nc.vector.tensor_tensor(
                t4[:], t3[:, :2 * B], t3[:, 2 * B:], op=mybir.AluOpType.add)
            z = bigpool.tile([128, B], _f32, name=f"z_{h}")
            nc.vector.tensor_tensor(
                z[:], t4[:, :B], t4[:, B:], op=mybir.AluOpType.add)
            zr = bigpool.tile([128, B], _f32, name=f"zr_{h}")
            nc.vector.reciprocal(zr[:], z[:])
            zrb = bigpool.tile([128, B], _bf16)
            nc.scalar.copy(zrb[:], zr[:])
            nc.vector.tensor_tensor(
                xh[h][:].rearrange("i (p b) -> i p b", p=P, b=B),
                xi_t[h][:].rearrange("i (p b) -> i p b", p=P, b=B),
                zrb[:].unsqueeze(1).broadcast_to([128, P, B]),
                op=mybir.AluOpType.mult,
            )

        def phase2_j(h, j, pool=False):
            """cx = e_j * xh ; s += cx^T W for one (h, j)."""
            cx = cpool.tile([128, P * B], _bf16)
            eng = nc.gpsimd if pool else nc.vector
            eng.tensor_tensor(
                cx[:].rearrange("i (p b) -> i p b", p=P, b=B),
                eT[h][:, j * B:(j + 1) * B].unsqueeze(1)
                    .broadcast_to([128, P, B]),
                xh[h][:].rearrange("i (p b) -> i p b", p=P, b=B),
                op=mybir.AluOpType.mult,
            )
            for p in range(P):
                nc.tensor.matmul(
                    s_ps[:, j * D:(j + 1) * D],
                    lhsT=cx[:, p * B:(p + 1) * B],
                    rhs=w4_t[h][:, (p * J + j) * D:(p * J + j + 1) * D],
                    start=False,
                    stop=(h == N_H - 1 and p == P - 1),
                    skip_group_check=True,
                )

        # ---- schedule: P1(h0) | P1(h1) interleaved with P2(h0) | P2(h1)
        phase1_chunk(0, MIX_A)
        znorm(0)
        phase1_chunk(1, MIX_B,
                     interleave=lambda j: phase2_j(
                         0, j, pool=(CX_B[j % len(CX_B)] == "p")))
        znorm(1)
        for j in range(J):
            phase2_j(1, j, pool=(CX_C[j % len(CX_C)] == "p"))

        sout = bigpool.tile([B, JD], _f32)
        nc.scalar.copy(sout[:], s_ps[:])
        nc.sync.dma_start(sp[:], sout[:])
    return nc


# ---------------------------------------------------------------------------
# Host glue
def _squash(s):
    v = s.reshape(B, J, D)
    s2 = np.sum(np.square(v), axis=-1, keepdims=True)
    scale = s2 / (1.0 + s2) / np.sqrt(s2 + EPS)
    return (scale * v).astype(np.float32)


_cache = {}


def _get_nc(name):
    if name not in _cache:
        _cache[name] = build_l1() if name == "l1" else build_l2()
    return _cache[name]


def _prep_inputs(x, W):
    """Per-core host-side re-layouts (bf16)."""
    sel = np.zeros((128, 8 * 128), _bf)
    for cc in range(8):
        for io in range(16):
            sel[io * 8:(io + 1) * 8, cc * 128 + cc * 16 + io] = 1.0
    per_core = []
    for c in range(N_CORES):
        sl = slice(c * I_LOC, (c + 1) * I_LOC)
        xc = x[:, sl, :]                                   # [B, 256, P]
        wc = W[:, sl, :, :]                                # [J, 256, D, P]
        # xi[h, il, p*B+b] = x[b, 128h+il, p]
        xi = np.ascontiguousarray(
            xc.transpose(1, 2, 0).reshape(N_H, 128, P * B)).astype(_bf)
        # xti[io*8+p, c*B+b] = x[b, 16c+io, p]
        xti = np.ascontiguousarray(
            xc.reshape(B, 16, 16, P).transpose(2, 3, 1, 0)
              .reshape(128, 16 * B)).astype(_bf)
        # w4[h, il, (p*J+j)*D+d] = W[j, 128h+il, d, p]
        w4 = np.ascontiguousarray(
            wc.transpose(1, 3, 0, 2).reshape(N_H, 128, P * JD)).astype(_bf)
        # w2t[jh, jm*16+d, c*128 + io*8+p] = W[8jh+jm, 16c+io, d, p]
        w2 = (wc.reshape(JH, 8, 16, 16, D, P)      # [jh, jm, c, io, d, p]
                .transpose(0, 1, 4, 2, 3, 5)       # [jh, jm, d, c, io, p]
                .reshape(JH, 128, 16 * 128).astype(_bf))
        per_core.append(
            {"xi": xi, "xti": xti, "w4": w4, "sel": sel,
             "w2t": np.ascontiguousarray(w2)})
    return per_core


def _ot_layout(O):
    """O [B,J,D] f32 -> otd[jh, jm*16+d, jm2*B+b] = O[b,8jh+jm,d] iff jm==jm2."""
    o4 = O.reshape(B, JH, 8, D).transpose(1, 2, 3, 0)   # [jh, jm, d, b]
    z = np.zeros((JH, 8, D, 8, B), np.float32)
    for jm in range(8):
        z[:, jm, :, jm, :] = o4[:, jm]
    return z.reshape(JH, 128, 8 * B).astype(_bf)


def _run(nc, in_maps, **kw):
    return run_bass_kernel_spmd(nc, in_maps, list(range(N_CORES)), **kw)


def kernel(x, W, _collect_times=None):
    x = np.asarray(x, dtype=np.float32)
    W = np.asarray(W, dtype=np.float32)
    pc = _prep_inputs(x, W)

    nc1 = _get_nc("l1")
    nc2 = _get_nc("l2")

    r1 = _run(nc1, [{"xi": p["xi"], "w4": p["w4"]} for p in pc])
    s0 = np.sum([r1.results[c]["sp"] for c in range(N_CORES)], axis=0,
                dtype=np.float64).astype(np.float32)
    s0 *= (1.0 / J)
    out0 = _squash(s0)

    ot1 = _ot_layout(out0)
    l2_maps = [
        {k: p[k] for k in ("xi", "xti", "w4", "sel", "w2t")}
        for p in pc
    ]
    r2 = _run(nc2, [{**m, "otd": ot1} for m in l2_maps])
    s1 = np.sum([r2.results[c]["sp"] for c in range(N_CORES)], axis=0,
                dtype=np.float64).astype(np.float32)
    out1 = _squash(s1)
    ocum = out0 + out1

    ot2 = _ot_layout(ocum)
    r3 = _run(nc2, [{**m, "otd": ot2} for m in l2_maps])
    s2 = np.sum([r3.results[c]["sp"] for c in range(N_CORES)], axis=0,
                dtype=np.float64).astype(np.float32)
    out2 = _squash(s2)

    if _collect_times is not None:
        for r in (r1, r2, r3):
            _collect_times.append(r.exec_time_ns)
    return out2


# revision 27
# speedup vs baseline: 4.1293x; 1.3429x over previous
"""CapsuleLayer dynamic-routing kernel for 8 Trainium2 NeuronCores.

Problem: x [128, 2048, 8], W [32, 2048, 16, 8] (fp32)
  u_hat[b,j,i,d] = sum_p W[j,i,d,p] * x[b,i,p]
  3 rounds of routing-by-agreement (softmax over j, squash).
  b_k = (sum_{m<k} out_m) . u_hat, so each round is a streaming pass
  over i needing only O_cum = sum of previous outputs.

Sharding: i (input capsules) split 8 ways; every core holds the full
batch B=128. Per-round partial sums s[b,(j,d)] are reduced on the host
between the three launches (squash also on host, it's tiny).

Per-round dataflow on each core (transposed [i, (p,b)] layout, bf16):
  phase 1 (per j, per i-chunk h of 128):
    A[(io,p), b]  = sum_d W[j,i,d,p] O[b,j,d]      (PE, contract d=16)
    eA            = A (PSUM f32 -> SBUF bf16)      (Act copy)
    m             = eA * x                         (DVE bf16 2x)
    g[i, b]       = sum_p m                        (PE selector matmul)
    e             = exp(g)                         (Act, PSUM->SBUF bf16)
  softmax norm (per i-chunk): Z = sum_j e (Pool), Zr = 1/Z (DVE),
    xh = x * Zr (folded once, so c = e needs no extra pass)
  phase 2 (per j, per i-chunk):
    cx[i,(p,b)]   = e_j * xh                       (DVE/Pool bf16)
    s[b,(j,d)]   += cx^T W                         (PE, contract (i,p))
"""

import numpy as np
import ml_dtypes
from contextlib import ExitStack

import concourse.bass as bass
import concourse.mybir as mybir
from concourse import tile
from concourse.bass_utils import run_bass_kernel_spmd

# ---------------------------------------------------------------------------
# Shapes (hardcoded for this problem)
B, I, P = 128, 2048, 8
J, D = 32, 16
JD = J * D               # 512
N_CORES = 8
I_LOC = I // N_CORES     # 256
EPS = 1e-7

_f32 = mybir.dt.float32
_bf16 = mybir.dt.bfloat16
_bf = ml_dtypes.bfloat16

N_H = 2                  # i-chunks of 128 per core
JH = J // 8              # 4  (j // 8 index in w2t/otd layouts)

# Per-section engine assignment patterns (index j % len):
#   mult paths: 'a' = Act evac + DVE bf16 mult, 'd' = DVE direct from PSUM,
#               'p' = Pool direct from PSUM
#   cx paths:   'd' = DVE, 'p' = Pool
GSZ = 4                  # j's per G/exp group
MIX_A = "aadaadaa"       # section 1 mults (P1 h0 alone)
MIX_B = "aaadaqad"       # section 2 mults (P1 h1 + P2 h0)
CX_B = "ddpd"           # section 2 cx
CX_C = "ddddp"          # tail cx


# ---------------------------------------------------------------------------
# Walrus compat: this toolchain rejects sync waits on InstDrain and >2 on
# InstEventSemaphore. Emit the waits as standalone nops before the drain.
def _apply_tile_compat():
    from concourse.vector_clock import ScopedClock

    def _strip_waits(inst):
        si = inst.sync_info
        if not si or not si.on_wait:
            return []
        waits = list(si.on_wait)
        si.on_wait = []
        inst.sync_info = si
        return waits

    def _nop_with_wait(eng, w):
        nop = eng.nop(nofuse=True, hint="drain_wait_split")
        nsi = nop.ins.sync_info
        if nsi is None:
            nsi = mybir.SyncInfo(on_wait=[], on_update=[])
        nsi.on_wait = list(nsi.on_wait or []) + [w]
        nop.ins.sync_info = nsi

    def _patched_multi_engine_barrier(self, engines):
        for inst in bass._bass_rust._multi_engine_barrier_insts(
            self, list(engines)
        ):
            eng = self.engines[inst.engine]
            for w in _strip_waits(inst):
                _nop_with_wait(eng, w)
            eng.add_instruction(inst)

    def _patched_drain_and_barrier(self, tick_clock, wait_clock):
        nop_inst = self.nc.sync.nop(nofuse=True, hint="drain_wait_split")
        wait_clock.add_sem_waits(
            nop_inst.ins, ScopedClock({None: tick_clock.global_clock})
        )
        si = nop_inst.ins.sync_info
        if si and si.on_wait and len(si.on_wait) > 1:
            extra = list(si.on_wait[1:])
            si.on_wait = [si.on_wait[0]]
            nop_inst.ins.sync_info = si
            for w in extra:
                _nop_with_wait(self.nc.sync, w)
        self.nc.sync.drain()

        self.nc.all_engine_barrier()
        assert self.sems is not None
        popped = self.nc._tile_sem_poison_stack.pop()
        assert popped is self._sem_poison
        self.nc.clear_and_free_semaphores(list(self.sems.allocated().values()))
        # No trailing all_engine_barrier: every engine is already past the
        # pre-clear barrier (done touching semaphores), nothing reads them
        # afterwards, and NEFF completion only needs each engine to halt.

    # Scheduled body instructions can also end up with >1 wait (e.g. a
    # matmul waiting on two DMAs). Spill extras onto same-engine NoOps
    # inserted immediately before the instruction.
    _WAIT_CAPS = {"InstDrain": 0, "InstEventSemaphore": 2}
    _orig_add_instruction = tile.TileContext._add_instruction

    def _patched_add_instruction(self, inst):
        si = inst.sync_info
        cap = _WAIT_CAPS.get(type(inst).__name__, 1)
        if si and si.on_wait and len(si.on_wait) > cap:
            waits = list(si.on_wait)
            si.on_wait = waits[:cap]
            inst.sync_info = si
            for w in waits[cap:]:
                nop = mybir.InstNoOp(
                    name=f"I-{self.nc.next_id()}-waitspill", ins=[], outs=[]
                )
                nop.engine = inst.engine
                nop.sync_info = mybir.SyncInfo(on_wait=[w], on_update=[])
                _orig_add_instruction(self, nop)
        _orig_add_instruction(self, inst)

    bass.Bass.multi_engine_barrier = _patched_multi_engine_barrier
    tile.TileContext._drain_and_barrier = _patched_drain_and_barrier
    tile.TileContext._add_instruction = _patched_add_instruction


_apply_tile_compat()


# ---------------------------------------------------------------------------
# Launch 1 (round 0): s0_part[b,(j,d)] = sum_{i local} u_hat[b,j,i,d]
# (iteration 0 has exactly uniform c = 1/32, applied on the host)
def build_l1():
    nc = bass.Bass("TRN2", target_bir_lowering=False, debug=False)
    xi = nc.dram_tensor("xi", [N_H, 128, P * B], _bf16, kind="ExternalInput").ap()
    w4 = nc.dram_tensor("w4", [N_H, 128, P * JD], _bf16, kind="ExternalInput").ap()
    sp = nc.dram_tensor("sp", [B, JD], _f32, kind="ExternalOutput").ap()
    with ExitStack() as ctx:
        tc = ctx.enter_context(tile.TileContext(nc))
        xpool = ctx.enter_context(tc.tile_pool(name="x", bufs=2))
        wpool = ctx.enter_context(tc.tile_pool(name="w", bufs=2))
        ppool = ctx.enter_context(tc.tile_pool(name="ps", bufs=1, space="PSUM"))
        opool = ctx.enter_context(tc.tile_pool(name="o", bufs=1))
        xs = []
        for h in range(N_H):
            xt = xpool.tile([128, P * B], _bf16)
            nc.sync.dma_start(xt[:], xi[h])
            xs.append(xt)
        psum = ppool.tile([B, JD], _f32)
        wts = []
        Q = P * JD // 4
        for h in range(N_H):
            wt = wpool.tile([128, P * JD], _bf16)
            # quarter-split the W load so the first matmuls start sooner
            for q in range(4):
                nc.sync.dma_start(wt[:, q * Q:(q + 1) * Q],
                                  w4[h][:, q * Q:(q + 1) * Q])
            wts.append(wt)
        # warm the PE while DMAs land; opens the psum epoch with zeros
        wup = xpool.tile([128, 512], _bf16, name="wup")
        nc.vector.memset(wup[:], 0.0)
        nc.tensor.matmul(psum[:], lhsT=wup[:, :128], rhs=wup[:],
                         start=True, stop=False, skip_group_check=True)
        for _ in range(6):
            nc.tensor.matmul(psum[:], lhsT=wup[:, :128], rhs=wup[:],
                             start=False, stop=False, skip_group_check=True)
        for h in range(N_H):
            wt = wts[h]
            for p in range(P):
                nc.tensor.matmul(
                    psum[:],
                    lhsT=xs[h][:, p * B:(p + 1) * B],
                    rhs=wt[:, p * JD:(p + 1) * JD],
                    start=False,
                    stop=(h == N_H - 1 and p == P - 1),
                    skip_group_check=True,
                )
        out = opool.tile([B, JD], _f32)
        nc.scalar.copy(out[:], psum[:])
        nc.sync.dma_start(sp[:], out[:])
    return nc


# ---------------------------------------------------------------------------
# Launches 2 & 3: one routing round (see module docstring).
def build_l2():
    nc = bass.Bass("TRN2", target_bir_lowering=False, debug=False)
    # Stage-1 contracts over the full 128 partitions (jm, d); the otd rhs
    # is zero outside the target j's 16 d-rows, so base partitions stay 0.
    # w2t[jh][jm*16+d, c*128 + io*8+p] = W[8jh+jm, 16c+io, d, p]
    w2t = nc.dram_tensor("w2t", [JH, 128, 16 * 128], _bf16,
                         kind="ExternalInput").ap()
    w4 = nc.dram_tensor("w4", [N_H, 128, P * JD], _bf16, kind="ExternalInput").ap()
    xti = nc.dram_tensor("xti", [128, 16 * B], _bf16, kind="ExternalInput").ap()
    xi = nc.dram_tensor("xi", [N_H, 128, P * B], _bf16, kind="ExternalInput").ap()
    # otd[jh][jm*16+d, jm2*B + b] = O[b, 8jh+jm2, d] if jm==jm2 else 0
    otd = nc.dram_tensor("otd", [JH, 128, 8 * B], _bf16,
                         kind="ExternalInput").ap()
    # sel[io*8+p, cc*128 + cc*16+io] = 1 (one selector block per cc)
    sel = nc.dram_tensor("sel", [128, 8 * 128], _bf16, kind="ExternalInput").ap()
    sp = nc.dram_tensor("sp", [B, JD], _f32, kind="ExternalOutput").ap()

    with ExitStack() as ctx:
        tc = ctx.enter_context(tile.TileContext(nc))
        inpool = ctx.enter_context(tc.tile_pool(name="in", bufs=1))
        apool = ctx.enter_context(tc.tile_pool(name="aps", bufs=2, space="PSUM"))
        gpool = ctx.enter_context(tc.tile_pool(name="gps", bufs=2, space="PSUM"))
        spool = ctx.enter_context(tc.tile_pool(name="sps", bufs=1, space="PSUM"))
        eapool = ctx.enter_context(tc.tile_pool(name="ea", bufs=4))
        mpool = ctx.enter_context(tc.tile_pool(name="m", bufs=8))
        cpool = ctx.enter_context(tc.tile_pool(name="cx", bufs=4))
        bigpool = ctx.enter_context(tc.tile_pool(name="big", bufs=1))
        zppool = ctx.enter_context(tc.tile_pool(name="zp", bufs=14))

        # ---- input loads (order matters for the pipeline head)
        w2t_t = []
        otd_t = []
        for jh in range(JH):
            t = inpool.tile([128, 16 * 128], _bf16, name=f"w2t_t{jh}")
            w2t_t.append(t)
            t2 = inpool.tile([128, 8 * B], _bf16, name=f"otd_t{jh}")
            otd_t.append(t2)
        sel_t = inpool.tile([128, 8 * 128], _bf16)
        xti_t = inpool.tile([128, 16 * B], _bf16)
        with tc.high_priority():
            nc.scalar.dma_start(otd_t[0][:], otd[0])
            nc.sync.dma_start(w2t_t[0][:], w2t[0])
            nc.scalar.dma_start(xti_t[:], xti[:])
            nc.sync.dma_start(sel_t[:], sel[:])
        for jh in range(1, JH):
            nc.sync.dma_start(w2t_t[jh][:], w2t[jh])
            nc.scalar.dma_start(otd_t[jh][:], otd[jh])

        # PE warm-up: ramp the tensor engine to full clock while the first
        # DMAs land (p-state model needs ~3us of continuous activity).
        wup = inpool.tile([128, 128], _bf16)
        nc.vector.memset(wup[:], 0.0)
        wu_ps = spool.tile([B, 512], _f32, name="wu_ps")
        for _ in range(10):
            nc.tensor.matmul(wu_ps[:, :128], lhsT=wup[:], rhs=wup[:],
                             start=True, stop=True)
        xi_t = []
        for h in range(N_H):
            t = inpool.tile([128, P * B], _bf16, name=f"xi_t{h}")
            nc.scalar.dma_start(t[:], xi[h])
            xi_t.append(t)
        w4_t = []
        for h in range(N_H):
            t = inpool.tile([128, P * JD], _bf16, name=f"w4_t{h}")
            nc.scalar.dma_start(t[:], w4[h])
            w4_t.append(t)

        eT = [bigpool.tile([128, J * B], _bf16, name=f"eT{h}") for h in range(N_H)]
        zparts = {0: [], 1: []}
        xh = [bigpool.tile([128, P * B], _bf16, name=f"xh{h}") for h in range(N_H)]

        def phase1_j(h, j, path):
            """A = W.O ; m = A*x for one (h, j)."""
            jm, jh = j % 8, j // 8
            a_ps = apool.tile([128, 8 * B], _f32)
            for cc in range(8):
                c = h * 8 + cc
                nc.tensor.matmul(
                    a_ps[:, cc * B:(cc + 1) * B],
                    lhsT=w2t_t[jh][:, c * 128:(c + 1) * 128],
                    rhs=otd_t[jh][:, jm * B:(jm + 1) * B],
                    start=True, stop=True,
                )
            m = mpool.tile([128, 8 * B], _bf16)
            xsl = xti_t[:, h * 8 * B:(h + 1) * 8 * B]
            if path == "d":
                nc.vector.tensor_tensor(
                    m[:], a_ps[:], xsl, op=mybir.AluOpType.mult)
            else:
                ea = eapool.tile([128, 8 * B], _bf16)
                nc.scalar.copy(ea[:], a_ps[:])
                eng = nc.gpsimd if path == "q" else nc.vector
                eng.tensor_tensor(
                    m[:], ea[:], xsl, op=mybir.AluOpType.mult)
            return m

        def sel_reduce(g_ps, m, jj):
            # g[:, jj] = sum_cc SEL_cc^T m_cc  (accumulating, full M=128)
            for cc in range(8):
                nc.tensor.matmul(
                    g_ps[:, jj * B:(jj + 1) * B],
                    lhsT=sel_t[:, cc * 128:(cc + 1) * 128],
                    rhs=m[:, cc * B:(cc + 1) * B],
                    start=(cc == 0), stop=(cc == 7),
                )

        # Software-pipelined phase 1 over one i-chunk h: stage1/evac/mult
        # for j runs STAGGER j-slots ahead of the SEL reduce for j, so the
        # PE never blocks on the Act->DVE chain. interleave(j) lets the
        # caller splice phase-2 work of the other chunk into each slot.
        STAGGER = 2

        def phase1_chunk(h, mix, interleave=None):
            pending = []   # (m_tile, j)
            g_tiles = {}
            for j in range(J + STAGGER):
                if j < J:
                    m = phase1_j(h, j, mix[j % len(mix)])
                    pending.append((m, j))
                if j >= STAGGER:
                    m0, j0 = pending.pop(0)
                    if j0 % GSZ == 0:
                        g_tiles[j0 // GSZ] = gpool.tile(
                            [128, GSZ * B], _f32, name="g_ps")
                    sel_reduce(g_tiles[j0 // GSZ], m0, j0 % GSZ)
                    if j0 % GSZ == GSZ - 1:
                        jg = j0 // GSZ
                        esl = eT[h][:, jg * GSZ * B:(jg + 1) * GSZ * B]
                        nc.scalar.activation(
                            esl, g_tiles.pop(jg)[:],
                            mybir.ActivationFunctionType.Exp,
                        )

                if interleave is not None and j < J:
                    interleave(j)

        def znorm(h):
            """Z = sum_j e (bf16 add tree over j-blocks); xh = x * (1/Z)."""
            e = eT[h]
            t1 = bigpool.tile([128, 16 * B], _bf16, name=f"zt1_{h}")
            nc.vector.tensor_tensor(
                t1[:], e[:, :16 * B], e[:, 16 * B:], op=mybir.AluOpType.add)
            t2 = bigpool.tile([128, 8 * B], _bf16, name=f"zt2_{h}")
            nc.vector.tensor_tensor(
                t2[:], t1[:, :8 * B], t1[:, 8 * B:], op=mybir.AluOpType.add)
            t3 = bigpool.tile([128, 4 * B], _bf16, name=f"zt3_{h}")
            nc.vector.tensor_tensor(
                t3[:], t2[:, :4 * B], t2[:, 4 * B:], op=mybir.AluOpType.add)
            t4 = bigpool.tile([128, 2 * B], _bf16, name=f"zt4_{h}")
            nc.vector.tensor_tensor(
                t4[:], t3[:, :2 * B], t3[:, 2 * B:], op=mybir.AluOpType.add)
            z = bigpool.tile([128, B], _f32, name=f"z_{h}")
            nc.vector.tensor_tensor(
                z[:], t4[:, :B], t4[:, B:], op=mybir.AluOpType.add)
            zr = bigpool.tile([128, B], _f32, name=f"zr_{h}")
            nc.vector.reciprocal(zr[:], z[:])
            zrb = bigpool.tile([128, B], _bf16)
            nc.scalar.copy(zrb[:], zr[:])
            nc.vector.tensor_tensor(
                xh[h][:].rearrange("i (p b) -> i p b", p=P, b=B),
                xi_t[h][:].rearrange("i (p b) -> i p b", p=P, b=B),
                zrb[:].unsqueeze(1).broadcast_to([128, P, B]),
                op=mybir.AluOpType.mult,
            )

        def phase2_j(h, j, pool=False):
            """cx = e_j * xh for a PAIR (j, j+1) in one instr; s += cx^T W."""
            cx = cpool.tile([128, 2 * P * B], _bf16)
            eng = nc.gpsimd if pool else nc.vector
            eng.tensor_tensor(
                cx[:].rearrange("i (j p b) -> i j p b", j=2, p=P, b=B),
                eT[h][:, j * B:(j + 2) * B]
                    .rearrange("i (j b) -> i j b", j=2, b=B)
                    .unsqueeze(2).broadcast_to([128, 2, P, B]),
                xh[h][:].rearrange("i (p b) -> i p b", p=P, b=B)
                    .unsqueeze(1).broadcast_to([128, 2, P, B]),
                op=mybir.AluOpType.mult,
            )
            for jj in range(2):
                for p in range(P):
                    nc.tensor.matmul(
                        s_ps[:, (j + jj) * D:(j + jj + 1) * D],
                        lhsT=cx[:, (jj * P + p) * B:(jj * P + p + 1) * B],
                        rhs=w4_t[h][:, (p * J + j + jj) * D:
                                    (p * J + j + jj + 1) * D],
                        start=False,
                        stop=(h == N_H - 1 and p == P - 1),
                        skip_group_check=True,
                    )

        # ---- schedule: P1(h0) | P1(h1) interleaved with P2(h0) | P2(h1)
        phase1_chunk(0, MIX_A)
        znorm(0)
        phase1_chunk(1, MIX_B,
                     interleave=lambda j: (phase2_j(
                         0, j, pool=(CX_B[(j // 2) % len(CX_B)] == "p"))
                         if j % 2 == 0 else None))
        znorm(1)
        for j in range(0, J, 2):
            phase2_j(1, j, pool=(CX_C[(j // 2) % len(CX_C)] == "p"))

        sout = bigpool.tile([B, JD], _f32)
        nc.scalar.copy(sout[:], s_ps[:])
        nc.sync.dma_start(sp[:], sout[:])
    return nc


# ---------------------------------------------------------------------------
# Host glue
def _squash(s):
    v = s.reshape(B, J, D)
    s2 = np.sum(np.square(v), axis=-1, keepdims=True)
    scale = s2 / (1.0 + s2) / np.sqrt(s2 + EPS)
    return (scale * v).astype(np.float32)


_cache = {}


def _get_nc(name):
    if name not in _cache:
        _cache[name] = build_l1() if name == "l1" else build_l2()
    return _cache[name]


def _prep_inputs(x, W):
    """Per-core host-side re-layouts (bf16)."""
    sel = np.zeros((128, 8 * 128), _bf)
    for cc in range(8):
        for io in range(16):
            sel[io * 8:(io + 1) * 8, cc * 128 + cc * 16 + io] = 1.0
    per_core = []
    for c in range(N_CORES):
        sl = slice(c * I_LOC, (c + 1) * I_LOC)
        xc = x[:, sl, :]                                   # [B, 256, P]
        wc = W[:, sl, :, :]                                # [J, 256, D, P]
        # xi[h, il, p*B+b] = x[b, 128h+il, p]
        xi = np.ascontiguousarray(
            xc.transpose(1, 2, 0).reshape(N_H, 128, P * B)).astype(_bf)
        # xti[io*8+p, c*B+b] = x[b, 16c+io, p]
        xti = np.ascontiguousarray(
            xc.reshape(B, 16, 16, P).transpose(2, 3, 1, 0)
              .reshape(128, 16 * B)).astype(_bf)
        # w4[h, il, (p*J+j)*D+d] = W[j, 128h+il, d, p]
        w4 = np.ascontiguousarray(
            wc.transpose(1, 3, 0, 2).reshape(N_H, 128, P * JD)).astype(_bf)
        # w2t[jh, jm*16+d, c*128 + io*8+p] = W[8jh+jm, 16c+io, d, p]
        w2 = (wc.reshape(JH, 8, 16, 16, D, P)      # [jh, jm, c, io, d, p]
                .transpose(0, 1, 4, 2, 3, 5)       # [jh, jm, d, c, io, p]
                .reshape(JH, 128, 16 * 128).astype(_bf))
        per_core.append(
            {"xi": xi, "xti": xti, "w4": w4, "sel": sel,
             "w2t": np.ascontiguousarray(w2)})
    return per_core


def _ot_layout(O):
    """O [B,J,D] f32 -> otd[jh, jm*16+d, jm2*B+b] = O[b,8jh+jm,d] iff jm==jm2."""
    o4 = O.reshape(B, JH, 8, D).transpose(1, 2, 3, 0)   # [jh, jm, d, b]
    z = np.zeros((JH, 8, D, 8, B), np.float32)
    for jm in range(8):
        z[:, jm, :, jm, :] = o4[:, jm]
    return z.reshape(JH, 128, 8 * B).astype(_bf)


def _run(nc, in_maps, **kw):
    return run_bass_kernel_spmd(nc, in_maps, list(range(N_CORES)), **kw)


def kernel(x, W, _collect_times=None):
    x = np.asarray(x, dtype=np.float32)
    W = np.asarray(W, dtype=np.float32)
    pc = _prep_inputs(x, W)

    nc1 = _get_nc("l1")
    nc2 = _get_nc("l2")

    r1 = _run(nc1, [{"xi": p["xi"], "w4": p["w4"]} for p in pc])
    s0 = np.sum([r1.results[c]["sp"] for c in range(N_CORES)], axis=0,
                dtype=np.float64).astype(np.float32)
    s0 *= (1.0 / J)
    out0 = _squash(s0)

    ot1 = _ot_layout(out0)
    l2_maps = [
        {k: p[k] for k in ("xi", "xti", "w4", "sel", "w2t")}
        for p in pc
    ]
    r2 = _run(nc2, [{**m, "otd": ot1} for m in l2_maps])
    s1 = np.sum([r2.results[c]["sp"] for c in range(N_CORES)], axis=0,
                dtype=np.float64).astype(np.float32)
    out1 = _squash(s1)
    ocum = out0 + out1

    ot2 = _ot_layout(ocum)
    r3 = _run(nc2, [{**m, "otd": ot2} for m in l2_maps])
    s2 = np.sum([r3.results[c]["sp"] for c in range(N_CORES)], axis=0,
                dtype=np.float64).astype(np.float32)
    out2 = _squash(s2)

    if _collect_times is not None:
        for r in (r1, r2, r3):
            _collect_times.append(r.exec_time_ns)
    return out2


# revision 28
# speedup vs baseline: 4.1792x; 1.0121x over previous
"""CapsuleLayer dynamic-routing kernel for 8 Trainium2 NeuronCores.

Problem: x [128, 2048, 8], W [32, 2048, 16, 8] (fp32)
  u_hat[b,j,i,d] = sum_p W[j,i,d,p] * x[b,i,p]
  3 rounds of routing-by-agreement (softmax over j, squash).
  b_k = (sum_{m<k} out_m) . u_hat, so each round is a streaming pass
  over i needing only O_cum = sum of previous outputs.

Sharding: i (input capsules) split 8 ways; every core holds the full
batch B=128. Per-round partial sums s[b,(j,d)] are reduced on the host
between the three launches (squash also on host, it's tiny).

Per-round dataflow on each core (transposed [i, (p,b)] layout, bf16):
  phase 1 (per j, per i-chunk h of 128):
    A[(io,p), b]  = sum_d W[j,i,d,p] O[b,j,d]      (PE, contract d=16)
    eA            = A (PSUM f32 -> SBUF bf16)      (Act copy)
    m             = eA * x                         (DVE bf16 2x)
    g[i, b]       = sum_p m                        (PE selector matmul)
    e             = exp(g)                         (Act, PSUM->SBUF bf16)
  softmax norm (per i-chunk): Z = sum_j e (Pool), Zr = 1/Z (DVE),
    xh = x * Zr (folded once, so c = e needs no extra pass)
  phase 2 (per j, per i-chunk):
    cx[i,(p,b)]   = e_j * xh                       (DVE/Pool bf16)
    s[b,(j,d)]   += cx^T W                         (PE, contract (i,p))
"""

import numpy as np
import ml_dtypes
from contextlib import ExitStack

import concourse.bass as bass
import concourse.mybir as mybir
from concourse import tile
from concourse.bass_utils import run_bass_kernel_spmd

# ---------------------------------------------------------------------------
# Shapes (hardcoded for this problem)
B, I, P = 128, 2048, 8
J, D = 32, 16
JD = J * D               # 512
N_CORES = 8
I_LOC = I // N_CORES     # 256
EPS = 1e-7

_f32 = mybir.dt.float32
_bf16 = mybir.dt.bfloat16
_bf = ml_dtypes.bfloat16

N_H = 2                  # i-chunks of 128 per core
JH = J // 8              # 4  (j // 8 index in w2t/otd layouts)

# Per-section engine assignment patterns (index j % len):
#   mult paths: 'a' = Act evac + DVE bf16 mult, 'd' = DVE direct from PSUM,
#               'p' = Pool direct from PSUM
#   cx paths:   'd' = DVE, 'p' = Pool
GSZ = 4                  # j's per G/exp group
MIX_A = "aadaadaa"       # section 1 mults (P1 h0 alone)
MIX_B = "aaadaqad"       # section 2 mults (P1 h1 + P2 h0)
CX_B = "ddpd"           # section 2 cx
CX_C = "ddddp"          # tail cx


# ---------------------------------------------------------------------------
# Walrus compat: this toolchain rejects sync waits on InstDrain and >2 on
# InstEventSemaphore. Emit the waits as standalone nops before the drain.
def _apply_tile_compat():
    from concourse.vector_clock import ScopedClock

    def _strip_waits(inst):
        si = inst.sync_info
        if not si or not si.on_wait:
            return []
        waits = list(si.on_wait)
        si.on_wait = []
        inst.sync_info = si
        return waits

    def _nop_with_wait(eng, w):
        nop = eng.nop(nofuse=True, hint="drain_wait_split")
        nsi = nop.ins.sync_info
        if nsi is None:
            nsi = mybir.SyncInfo(on_wait=[], on_update=[])
        nsi.on_wait = list(nsi.on_wait or []) + [w]
        nop.ins.sync_info = nsi

    def _patched_multi_engine_barrier(self, engines):
        for inst in bass._bass_rust._multi_engine_barrier_insts(
            self, list(engines)
        ):
            eng = self.engines[inst.engine]
            for w in _strip_waits(inst):
                _nop_with_wait(eng, w)
            eng.add_instruction(inst)

    def _patched_drain_and_barrier(self, tick_clock, wait_clock):
        nop_inst = self.nc.sync.nop(nofuse=True, hint="drain_wait_split")
        wait_clock.add_sem_waits(
            nop_inst.ins, ScopedClock({None: tick_clock.global_clock})
        )
        si = nop_inst.ins.sync_info
        if si and si.on_wait and len(si.on_wait) > 1:
            extra = list(si.on_wait[1:])
            si.on_wait = [si.on_wait[0]]
            nop_inst.ins.sync_info = si
            for w in extra:
                _nop_with_wait(self.nc.sync, w)
        self.nc.sync.drain()

        self.nc.all_engine_barrier()
        assert self.sems is not None
        popped = self.nc._tile_sem_poison_stack.pop()
        assert popped is self._sem_poison
        self.nc.clear_and_free_semaphores(list(self.sems.allocated().values()))
        # No trailing all_engine_barrier: every engine is already past the
        # pre-clear barrier (done touching semaphores), nothing reads them
        # afterwards, and NEFF completion only needs each engine to halt.

    # Scheduled body instructions can also end up with >1 wait (e.g. a
    # matmul waiting on two DMAs). Spill extras onto same-engine NoOps
    # inserted immediately before the instruction.
    _WAIT_CAPS = {"InstDrain": 0, "InstEventSemaphore": 2}
    _orig_add_instruction = tile.TileContext._add_instruction

    def _patched_add_instruction(self, inst):
        si = inst.sync_info
        cap = _WAIT_CAPS.get(type(inst).__name__, 1)
        if si and si.on_wait and len(si.on_wait) > cap:
            waits = list(si.on_wait)
            si.on_wait = waits[:cap]
            inst.sync_info = si
            for w in waits[cap:]:
                nop = mybir.InstNoOp(
                    name=f"I-{self.nc.next_id()}-waitspill", ins=[], outs=[]
                )
                nop.engine = inst.engine
                nop.sync_info = mybir.SyncInfo(on_wait=[w], on_update=[])
                _orig_add_instruction(self, nop)
        _orig_add_instruction(self, inst)

    bass.Bass.multi_engine_barrier = _patched_multi_engine_barrier
    tile.TileContext._drain_and_barrier = _patched_drain_and_barrier
    tile.TileContext._add_instruction = _patched_add_instruction


_apply_tile_compat()


# ---------------------------------------------------------------------------
# Launch 1 (round 0): s0_part[b,(j,d)] = sum_{i local} u_hat[b,j,i,d]
# (iteration 0 has exactly uniform c = 1/32, applied on the host)
def build_l1():
    nc = bass.Bass("TRN2", target_bir_lowering=False, debug=False)
    xi = nc.dram_tensor("xi", [N_H, 128, P * B], _bf16, kind="ExternalInput").ap()
    w4 = nc.dram_tensor("w4", [N_H, 128, P * JD], _bf16, kind="ExternalInput").ap()
    sp = nc.dram_tensor("sp", [B, JD], _f32, kind="ExternalOutput").ap()
    with ExitStack() as ctx:
        tc = ctx.enter_context(tile.TileContext(nc))
        xpool = ctx.enter_context(tc.tile_pool(name="x", bufs=2))
        wpool = ctx.enter_context(tc.tile_pool(name="w", bufs=2))
        ppool = ctx.enter_context(tc.tile_pool(name="ps", bufs=1, space="PSUM"))
        opool = ctx.enter_context(tc.tile_pool(name="o", bufs=1))
        xs = []
        for h in range(N_H):
            xt = xpool.tile([128, P * B], _bf16)
            nc.sync.dma_start(xt[:], xi[h])
            xs.append(xt)
        psum = ppool.tile([B, JD], _f32)
        wts = []
        Q = P * JD // 4
        for h in range(N_H):
            wt = wpool.tile([128, P * JD], _bf16)
            # quarter-split the W load so the first matmuls start sooner
            for q in range(4):
                nc.sync.dma_start(wt[:, q * Q:(q + 1) * Q],
                                  w4[h][:, q * Q:(q + 1) * Q])
            wts.append(wt)
        # warm the PE while DMAs land; opens the psum epoch with zeros
        wup = xpool.tile([128, 512], _bf16, name="wup")
        nc.vector.memset(wup[:], 0.0)
        nc.tensor.matmul(psum[:], lhsT=wup[:, :128], rhs=wup[:],
                         start=True, stop=False, skip_group_check=True)
        for _ in range(6):
            nc.tensor.matmul(psum[:], lhsT=wup[:, :128], rhs=wup[:],
                             start=False, stop=False, skip_group_check=True)
        for h in range(N_H):
            wt = wts[h]
            for p in range(P):
                nc.tensor.matmul(
                    psum[:],
                    lhsT=xs[h][:, p * B:(p + 1) * B],
                    rhs=wt[:, p * JD:(p + 1) * JD],
                    start=False,
                    stop=(h == N_H - 1 and p == P - 1),
                    skip_group_check=True,
                )
        out = opool.tile([B, JD], _f32)
        nc.scalar.copy(out[:], psum[:])
        nc.sync.dma_start(sp[:], out[:])
    return nc


# ---------------------------------------------------------------------------
# Launches 2 & 3: one routing round (see module docstring).
def build_l2():
    nc = bass.Bass("TRN2", target_bir_lowering=False, debug=False)
    # Stage-1 contracts over the full 128 partitions (jm, d); the otd rhs
    # is zero outside the target j's 16 d-rows, so base partitions stay 0.
    # w2t[jh][jm*16+d, c*128 + io*8+p] = W[8jh+jm, 16c+io, d, p]
    w2t = nc.dram_tensor("w2t", [JH, 128, 16 * 128], _bf16,
                         kind="ExternalInput").ap()
    w4 = nc.dram_tensor("w4", [N_H, 128, P * JD], _bf16, kind="ExternalInput").ap()
    xti = nc.dram_tensor("xti", [128, 16 * B], _bf16, kind="ExternalInput").ap()
    xi = nc.dram_tensor("xi", [N_H, 128, P * B], _bf16, kind="ExternalInput").ap()
    # otd[jh][jm*16+d, jm2*B + b] = O[b, 8jh+jm2, d] if jm==jm2 else 0
    otd = nc.dram_tensor("otd", [JH, 128, 8 * B], _bf16,
                         kind="ExternalInput").ap()
    # sel[io*8+p, cc*128 + cc*16+io] = 1 (one selector block per cc)
    sel = nc.dram_tensor("sel", [128, 8 * 128], _bf16, kind="ExternalInput").ap()
    sp = nc.dram_tensor("sp", [B, JD], _f32, kind="ExternalOutput").ap()

    with ExitStack() as ctx:
        tc = ctx.enter_context(tile.TileContext(nc))
        inpool = ctx.enter_context(tc.tile_pool(name="in", bufs=1))
        apool = ctx.enter_context(tc.tile_pool(name="aps", bufs=2, space="PSUM"))
        gpool = ctx.enter_context(tc.tile_pool(name="gps", bufs=2, space="PSUM"))
        spool = ctx.enter_context(tc.tile_pool(name="sps", bufs=1, space="PSUM"))
        eapool = ctx.enter_context(tc.tile_pool(name="ea", bufs=4))
        mpool = ctx.enter_context(tc.tile_pool(name="m", bufs=8))
        cpool = ctx.enter_context(tc.tile_pool(name="cx", bufs=4))
        bigpool = ctx.enter_context(tc.tile_pool(name="big", bufs=1))
        zppool = ctx.enter_context(tc.tile_pool(name="zp", bufs=14))

        # ---- input loads (order matters for the pipeline head)
        w2t_t = []
        otd_t = []
        for jh in range(JH):
            t = inpool.tile([128, 16 * 128], _bf16, name=f"w2t_t{jh}")
            w2t_t.append(t)
            t2 = inpool.tile([128, 8 * B], _bf16, name=f"otd_t{jh}")
            otd_t.append(t2)
        sel_t = inpool.tile([128, 8 * 128], _bf16)
        xti_t = inpool.tile([128, 16 * B], _bf16)
        with tc.high_priority():
            nc.scalar.dma_start(otd_t[0][:], otd[0])
            nc.sync.dma_start(w2t_t[0][:], w2t[0])
            nc.scalar.dma_start(xti_t[:], xti[:])
            nc.sync.dma_start(sel_t[:], sel[:])
        for jh in range(1, JH):
            nc.sync.dma_start(w2t_t[jh][:], w2t[jh])
            nc.scalar.dma_start(otd_t[jh][:], otd[jh])

        # PE warm-up: ramp the tensor engine to full clock while the first
        # DMAs land (p-state model needs ~3us of continuous activity).
        wup = inpool.tile([128, 128], _bf16)
        nc.vector.memset(wup[:], 0.0)
        wu_ps = spool.tile([B, 512], _f32, name="wu_ps")
        for _ in range(10):
            nc.tensor.matmul(wu_ps[:, :128], lhsT=wup[:], rhs=wup[:],
                             start=True, stop=True)
        xi_t = []
        for h in range(N_H):
            t = inpool.tile([128, P * B], _bf16, name=f"xi_t{h}")
            nc.scalar.dma_start(t[:], xi[h])
            xi_t.append(t)
        w4_t = []
        for h in range(N_H):
            t = inpool.tile([128, P * JD], _bf16, name=f"w4_t{h}")
            nc.scalar.dma_start(t[:], w4[h])
            w4_t.append(t)

        eT = [bigpool.tile([128, J * B], _bf16, name=f"eT{h}") for h in range(N_H)]
        zparts = {0: [], 1: []}
        xh = [bigpool.tile([128, P * B], _bf16, name=f"xh{h}") for h in range(N_H)]

        def phase1_j(h, j, path):
            """A = W.O ; m = A*x for one (h, j)."""
            jm, jh = j % 8, j // 8
            a_ps = apool.tile([128, 8 * B], _f32)
            for cc in range(8):
                c = h * 8 + cc
                nc.tensor.matmul(
                    a_ps[:, cc * B:(cc + 1) * B],
                    lhsT=w2t_t[jh][:, c * 128:(c + 1) * 128],
                    rhs=otd_t[jh][:, jm * B:(jm + 1) * B],
                    start=True, stop=True,
                )
            m = mpool.tile([128, 8 * B], _bf16)
            xsl = xti_t[:, h * 8 * B:(h + 1) * 8 * B]
            if path == "d":
                nc.vector.tensor_tensor(
                    m[:], a_ps[:], xsl, op=mybir.AluOpType.mult)
            else:
                ea = eapool.tile([128, 8 * B], _bf16)
                nc.scalar.copy(ea[:], a_ps[:])
                eng = nc.gpsimd if path == "q" else nc.vector
                eng.tensor_tensor(
                    m[:], ea[:], xsl, op=mybir.AluOpType.mult)
            return m

        def sel_reduce(g_ps, m, jj):
            # g[:, jj] = sum_cc SEL_cc^T m_cc  (accumulating, full M=128)
            for cc in range(8):
                nc.tensor.matmul(
                    g_ps[:, jj * B:(jj + 1) * B],
                    lhsT=sel_t[:, cc * 128:(cc + 1) * 128],
                    rhs=m[:, cc * B:(cc + 1) * B],
                    start=(cc == 0), stop=(cc == 7),
                )

        # Software-pipelined phase 1 over one i-chunk h: stage1/evac/mult
        # for j runs STAGGER j-slots ahead of the SEL reduce for j, so the
        # PE never blocks on the Act->DVE chain. interleave(j) lets the
        # caller splice phase-2 work of the other chunk into each slot.
        STAGGER = 2

        def phase1_chunk(h, mix, interleave=None):
            pending = []   # (m_tile, j)
            g_tiles = {}
            for j in range(J + STAGGER):
                if j < J:
                    m = phase1_j(h, j, mix[j % len(mix)])
                    pending.append((m, j))
                if j >= STAGGER:
                    m0, j0 = pending.pop(0)
                    if j0 % GSZ == 0:
                        g_tiles[j0 // GSZ] = gpool.tile(
                            [128, GSZ * B], _f32, name="g_ps")
                    sel_reduce(g_tiles[j0 // GSZ], m0, j0 % GSZ)
                    if j0 % GSZ == GSZ - 1:
                        jg = j0 // GSZ
                        esl = eT[h][:, jg * GSZ * B:(jg + 1) * GSZ * B]
                        nc.scalar.activation(
                            esl, g_tiles.pop(jg)[:],
                            mybir.ActivationFunctionType.Exp,
                        )

                if interleave is not None and j < J:
                    interleave(j)

        def znorm(h):
            """Z = sum_j e (bf16 add tree over j-blocks); xh = x * (1/Z)."""
            e = eT[h]
            t1 = bigpool.tile([128, 16 * B], _bf16, name=f"zt1_{h}")
            nc.vector.tensor_tensor(
                t1[:], e[:, :16 * B], e[:, 16 * B:], op=mybir.AluOpType.add)
            t2 = bigpool.tile([128, 8 * B], _bf16, name=f"zt2_{h}")
            nc.vector.tensor_tensor(
                t2[:], t1[:, :8 * B], t1[:, 8 * B:], op=mybir.AluOpType.add)
            t3 = bigpool.tile([128, 4 * B], _bf16, name=f"zt3_{h}")
            nc.vector.tensor_tensor(
                t3[:], t2[:, :4 * B], t2[:, 4 * B:], op=mybir.AluOpType.add)
            t4 = bigpool.tile([128, 2 * B], _bf16, name=f"zt4_{h}")
            nc.vector.tensor_tensor(
                t4[:], t3[:, :2 * B], t3[:, 2 * B:], op=mybir.AluOpType.add)
            z = bigpool.tile([128, B], _f32, name=f"z_{h}")
            nc.vector.tensor_tensor(
                z[:], t4[:, :B], t4[:, B:], op=mybir.AluOpType.add)
            zr = bigpool.tile([128, B], _f32, name=f"zr_{h}")
            nc.vector.reciprocal(zr[:], z[:])
            zrb = bigpool.tile([128, B], _bf16)
            nc.scalar.copy(zrb[:], zr[:])
            nc.vector.tensor_tensor(
                xh[h][:].rearrange("i (p b) -> i p b", p=P, b=B),
                xi_t[h][:].rearrange("i (p b) -> i p b", p=P, b=B),
                zrb[:].unsqueeze(1).broadcast_to([128, P, B]),
                op=mybir.AluOpType.mult,
            )

        def phase2_j(h, j, pool=False):
            """cx = e_j * xh for a PAIR (j, j+1) in one instr; s += cx^T W."""
            cx = cpool.tile([128, 2 * P * B], _bf16)
            eng = nc.gpsimd if pool else nc.vector
            eng.tensor_tensor(
                cx[:].rearrange("i (j p b) -> i j p b", j=2, p=P, b=B),
                eT[h][:, j * B:(j + 2) * B]
                    .rearrange("i (j b) -> i j b", j=2, b=B)
                    .unsqueeze(2).broadcast_to([128, 2, P, B]),
                xh[h][:].rearrange("i (p b) -> i p b", p=P, b=B)
                    .unsqueeze(1).broadcast_to([128, 2, P, B]),
                op=mybir.AluOpType.mult,
            )
            for jj in range(2):
                for p in range(P):
                    nc.tensor.matmul(
                        s_ps[:, (j + jj) * D:(j + jj + 1) * D],
                        lhsT=cx[:, (jj * P + p) * B:(jj * P + p + 1) * B],
                        rhs=w4_t[h][:, (p * J + j + jj) * D:
                                    (p * J + j + jj + 1) * D],
                        start=False,
                        stop=(h == N_H - 1 and p == P - 1),
                        skip_group_check=True,
                    )

        # ---- schedule: P1(h0) | P1(h1) interleaved with P2(h0) | P2(h1)
        phase1_chunk(0, MIX_A)
        znorm(0)
        phase1_chunk(1, MIX_B,
                     interleave=lambda j: (phase2_j(
                         0, j, pool=(CX_B[(j // 2) % len(CX_B)] == "p"))
                         if j % 2 == 0 else None))
        znorm(1)
        sout = bigpool.tile([B, JD], _f32)
        for j in range(0, J, 2):
            phase2_j(1, j, pool=(CX_C[(j // 2) % len(CX_C)] == "p"))
            if j % 8 == 6:
                # this 8-j block of s columns is final; ship it now
                q0, q1 = (j - 6) * D, (j + 2) * D
                nc.scalar.copy(sout[:, q0:q1], s_ps[:, q0:q1])
                nc.sync.dma_start(sp[:, q0:q1], sout[:, q0:q1])
    return nc


# ---------------------------------------------------------------------------
# Host glue
def _squash(s):
    v = s.reshape(B, J, D)
    s2 = np.sum(np.square(v), axis=-1, keepdims=True)
    scale = s2 / (1.0 + s2) / np.sqrt(s2 + EPS)
    return (scale * v).astype(np.float32)


_cache = {}


def _get_nc(name):
    if name not in _cache:
        _cache[name] = build_l1() if name == "l1" else build_l2()
    return _cache[name]


def _prep_inputs(x, W):
    """Per-core host-side re-layouts (bf16)."""
    sel = np.zeros((128, 8 * 128), _bf)
    for cc in range(8):
        for io in range(16):
            sel[io * 8:(io + 1) * 8, cc * 128 + cc * 16 + io] = 1.0
    per_core = []
    for c in range(N_CORES):
        sl = slice(c * I_LOC, (c + 1) * I_LOC)
        xc = x[:, sl, :]                                   # [B, 256, P]
        wc = W[:, sl, :, :]                                # [J, 256, D, P]
        # xi[h, il, p*B+b] = x[b, 128h+il, p]
        xi = np.ascontiguousarray(
            xc.transpose(1, 2, 0).reshape(N_H, 128, P * B)).astype(_bf)
        # xti[io*8+p, c*B+b] = x[b, 16c+io, p]
        xti = np.ascontiguousarray(
            xc.reshape(B, 16, 16, P).transpose(2, 3, 1, 0)
              .reshape(128, 16 * B)).astype(_bf)
        # w4[h, il, (p*J+j)*D+d] = W[j, 128h+il, d, p]
        w4 = np.ascontiguousarray(
            wc.transpose(1, 3, 0, 2).reshape(N_H, 128, P * JD)).astype(_bf)
        # w2t[jh, jm*16+d, c*128 + io*8+p] = W[8jh+jm, 16c+io, d, p]
        w2 = (wc.reshape(JH, 8, 16, 16, D, P)      # [jh, jm, c, io, d, p]
                .transpose(0, 1, 4, 2, 3, 5)       # [jh, jm, d, c, io, p]
                .reshape(JH, 128, 16 * 128).astype(_bf))
        per_core.append(
            {"xi": xi, "xti": xti, "w4": w4, "sel": sel,
             "w2t": np.ascontiguousarray(w2)})
    return per_core


def _ot_layout(O):
    """O [B,J,D] f32 -> otd[jh, jm*16+d, jm2*B+b] = O[b,8jh+jm,d] iff jm==jm2."""
    o4 = O.reshape(B, JH, 8, D).transpose(1, 2, 3, 0)   # [jh, jm, d, b]
    z = np.zeros((JH, 8, D, 8, B), np.float32)
    for jm in range(8):
        z[:, jm, :, jm, :] = o4[:, jm]
    return z.reshape(JH, 128, 8 * B).astype(_bf)


def _run(nc, in_maps, **kw):
    return run_bass_kernel_spmd(nc, in_maps, list(range(N_CORES)), **kw)


def kernel(x, W, _collect_times=None):
    x = np.asarray(x, dtype=np.float32)
    W = np.asarray(W, dtype=np.float32)
    pc = _prep_inputs(x, W)

    nc1 = _get_nc("l1")
    nc2 = _get_nc("l2")

    r1 = _run(nc1, [{"xi": p["xi"], "w4": p["w4"]} for p in pc])
    s0 = np.sum([r1.results[c]["sp"] for c in range(N_CORES)], axis=0,
                dtype=np.float64).astype(np.float32)
    s0 *= (1.0 / J)
    out0 = _squash(s0)

    ot1 = _ot_layout(out0)
    l2_maps = [
        {k: p[k] for k in ("xi", "xti", "w4", "sel", "w2t")}
        for p in pc
    ]
    r2 = _run(nc2, [{**m, "otd": ot1} for m in l2_maps])
    s1 = np.sum([r2.results[c]["sp"] for c in range(N_CORES)], axis=0,
                dtype=np.float64).astype(np.float32)
    out1 = _squash(s1)
    ocum = out0 + out1

    ot2 = _ot_layout(ocum)
    r3 = _run(nc2, [{**m, "otd": ot2} for m in l2_maps])
    s2 = np.sum([r3.results[c]["sp"] for c in range(N_CORES)], axis=0,
                dtype=np.float64).astype(np.float32)
    out2 = _squash(s2)

    if _collect_times is not None:
        for r in (r1, r2, r3):
            _collect_times.append(r.exec_time_ns)
    return out2


# revision 29
# speedup vs baseline: 4.1918x; 1.0030x over previous
"""CapsuleLayer dynamic-routing kernel for 8 Trainium2 NeuronCores.

Problem: x [128, 2048, 8], W [32, 2048, 16, 8] (fp32)
  u_hat[b,j,i,d] = sum_p W[j,i,d,p] * x[b,i,p]
  3 rounds of routing-by-agreement (softmax over j, squash).
  b_k = (sum_{m<k} out_m) . u_hat, so each round is a streaming pass
  over i needing only O_cum = sum of previous outputs.

Sharding: i (input capsules) split 8 ways; every core holds the full
batch B=128. Per-round partial sums s[b,(j,d)] are reduced on the host
between the three launches (squash also on host, it's tiny).

Per-round dataflow on each core (transposed [i, (p,b)] layout, bf16):
  phase 1 (per j, per i-chunk h of 128):
    A[(io,p), b]  = sum_d W[j,i,d,p] O[b,j,d]      (PE, contract d=16)
    eA            = A (PSUM f32 -> SBUF bf16)      (Act copy)
    m             = eA * x                         (DVE bf16 2x)
    g[i, b]       = sum_p m                        (PE selector matmul)
    e             = exp(g)                         (Act, PSUM->SBUF bf16)
  softmax norm (per i-chunk): Z = sum_j e (Pool), Zr = 1/Z (DVE),
    xh = x * Zr (folded once, so c = e needs no extra pass)
  phase 2 (per j, per i-chunk):
    cx[i,(p,b)]   = e_j * xh                       (DVE/Pool bf16)
    s[b,(j,d)]   += cx^T W                         (PE, contract (i,p))
"""

import numpy as np
import ml_dtypes
from contextlib import ExitStack

import concourse.bass as bass
import concourse.mybir as mybir
from concourse import tile
from concourse.bass_utils import run_bass_kernel_spmd

# ---------------------------------------------------------------------------
# Shapes (hardcoded for this problem)
B, I, P = 128, 2048, 8
J, D = 32, 16
JD = J * D               # 512
N_CORES = 8
I_LOC = I // N_CORES     # 256
EPS = 1e-7

_f32 = mybir.dt.float32
_bf16 = mybir.dt.bfloat16
_bf = ml_dtypes.bfloat16

N_H = 2                  # i-chunks of 128 per core
JH = J // 8              # 4  (j // 8 index in w2t/otd layouts)

# Per-section engine assignment patterns (index j % len):
#   mult paths: 'a' = Act evac + DVE bf16 mult, 'd' = DVE direct from PSUM,
#               'p' = Pool direct from PSUM
#   cx paths:   'd' = DVE, 'p' = Pool
GSZ = 4                  # j's per G/exp group
MIX_A = "aadaadaa"       # section 1 mults (P1 h0 alone)
MIX_B = "aaadaqad"       # section 2 mults (P1 h1 + P2 h0)
CX_B = "ddpd"           # section 2 cx
CX_C = "ddddp"          # tail cx


# ---------------------------------------------------------------------------
# Walrus compat: this toolchain rejects sync waits on InstDrain and >2 on
# InstEventSemaphore. Emit the waits as standalone nops before the drain.
def _apply_tile_compat():
    from concourse.vector_clock import ScopedClock

    def _strip_waits(inst):
        si = inst.sync_info
        if not si or not si.on_wait:
            return []
        waits = list(si.on_wait)
        si.on_wait = []
        inst.sync_info = si
        return waits

    def _nop_with_wait(eng, w):
        nop = eng.nop(nofuse=True, hint="drain_wait_split")
        nsi = nop.ins.sync_info
        if nsi is None:
            nsi = mybir.SyncInfo(on_wait=[], on_update=[])
        nsi.on_wait = list(nsi.on_wait or []) + [w]
        nop.ins.sync_info = nsi

    def _patched_multi_engine_barrier(self, engines):
        for inst in bass._bass_rust._multi_engine_barrier_insts(
            self, list(engines)
        ):
            eng = self.engines[inst.engine]
            for w in _strip_waits(inst):
                _nop_with_wait(eng, w)
            eng.add_instruction(inst)

    def _patched_drain_and_barrier(self, tick_clock, wait_clock):
        nop_inst = self.nc.sync.nop(nofuse=True, hint="drain_wait_split")
        wait_clock.add_sem_waits(
            nop_inst.ins, ScopedClock({None: tick_clock.global_clock})
        )
        si = nop_inst.ins.sync_info
        if si and si.on_wait and len(si.on_wait) > 1:
            extra = list(si.on_wait[1:])
            si.on_wait = [si.on_wait[0]]
            nop_inst.ins.sync_info = si
            for w in extra:
                _nop_with_wait(self.nc.sync, w)
        self.nc.sync.drain()

        self.nc.all_engine_barrier()
        assert self.sems is not None
        popped = self.nc._tile_sem_poison_stack.pop()
        assert popped is self._sem_poison
        self.nc.clear_and_free_semaphores(list(self.sems.allocated().values()))
        # No trailing all_engine_barrier: every engine is already past the
        # pre-clear barrier (done touching semaphores), nothing reads them
        # afterwards, and NEFF completion only needs each engine to halt.

    # Scheduled body instructions can also end up with >1 wait (e.g. a
    # matmul waiting on two DMAs). Spill extras onto same-engine NoOps
    # inserted immediately before the instruction.
    _WAIT_CAPS = {"InstDrain": 0, "InstEventSemaphore": 2}
    _orig_add_instruction = tile.TileContext._add_instruction

    def _patched_add_instruction(self, inst):
        si = inst.sync_info
        cap = _WAIT_CAPS.get(type(inst).__name__, 1)
        if si and si.on_wait and len(si.on_wait) > cap:
            waits = list(si.on_wait)
            si.on_wait = waits[:cap]
            inst.sync_info = si
            for w in waits[cap:]:
                nop = mybir.InstNoOp(
                    name=f"I-{self.nc.next_id()}-waitspill", ins=[], outs=[]
                )
                nop.engine = inst.engine
                nop.sync_info = mybir.SyncInfo(on_wait=[w], on_update=[])
                _orig_add_instruction(self, nop)
        _orig_add_instruction(self, inst)

    bass.Bass.multi_engine_barrier = _patched_multi_engine_barrier
    tile.TileContext._drain_and_barrier = _patched_drain_and_barrier
    tile.TileContext._add_instruction = _patched_add_instruction


_apply_tile_compat()


# ---------------------------------------------------------------------------
# Launch 1 (round 0): s0_part[b,(j,d)] = sum_{i local} u_hat[b,j,i,d]
# (iteration 0 has exactly uniform c = 1/32, applied on the host)
def build_l1():
    nc = bass.Bass("TRN2", target_bir_lowering=False, debug=False)
    xi = nc.dram_tensor("xi", [N_H, 128, P * B], _bf16, kind="ExternalInput").ap()
    w4 = nc.dram_tensor("w4", [N_H, 128, P * JD], _bf16, kind="ExternalInput").ap()
    sp = nc.dram_tensor("sp", [B, JD], _f32, kind="ExternalOutput").ap()
    with ExitStack() as ctx:
        tc = ctx.enter_context(tile.TileContext(nc))
        xpool = ctx.enter_context(tc.tile_pool(name="x", bufs=2))
        wpool = ctx.enter_context(tc.tile_pool(name="w", bufs=2))
        ppool = ctx.enter_context(tc.tile_pool(name="ps", bufs=1, space="PSUM"))
        opool = ctx.enter_context(tc.tile_pool(name="o", bufs=1))
        xs = []
        for h in range(N_H):
            xt = xpool.tile([128, P * B], _bf16)
            nc.sync.dma_start(xt[:], xi[h])
            xs.append(xt)
        psum = ppool.tile([B, JD], _f32)
        wts = []
        Q = P * JD // 4
        for h in range(N_H):
            wt = wpool.tile([128, P * JD], _bf16)
            # quarter-split the W load so the first matmuls start sooner
            for q in range(4):
                nc.sync.dma_start(wt[:, q * Q:(q + 1) * Q],
                                  w4[h][:, q * Q:(q + 1) * Q])
            wts.append(wt)
        # warm the PE while DMAs land; opens the psum epoch with zeros
        wup = xpool.tile([128, 512], _bf16, name="wup")
        nc.vector.memset(wup[:], 0.0)
        nc.tensor.matmul(psum[:], lhsT=wup[:, :128], rhs=wup[:],
                         start=True, stop=False, skip_group_check=True)
        for _ in range(6):
            nc.tensor.matmul(psum[:], lhsT=wup[:, :128], rhs=wup[:],
                             start=False, stop=False, skip_group_check=True)
        for h in range(N_H):
            wt = wts[h]
            for p in range(P):
                nc.tensor.matmul(
                    psum[:],
                    lhsT=xs[h][:, p * B:(p + 1) * B],
                    rhs=wt[:, p * JD:(p + 1) * JD],
                    start=False,
                    stop=(h == N_H - 1 and p == P - 1),
                    skip_group_check=True,
                )
        out = opool.tile([B, JD], _f32)
        nc.scalar.copy(out[:], psum[:])
        nc.sync.dma_start(sp[:], out[:])
    return nc


# ---------------------------------------------------------------------------
# Launches 2 & 3: one routing round (see module docstring).
def build_l2():
    nc = bass.Bass("TRN2", target_bir_lowering=False, debug=False)
    # Stage-1 contracts over the full 128 partitions (jm, d); the otd rhs
    # is zero outside the target j's 16 d-rows, so base partitions stay 0.
    # w2t[jh][jm*16+d, c*128 + io*8+p] = W[8jh+jm, 16c+io, d, p]
    w2t = nc.dram_tensor("w2t", [JH, 128, 16 * 128], _bf16,
                         kind="ExternalInput").ap()
    w4 = nc.dram_tensor("w4", [N_H, 128, P * JD], _bf16, kind="ExternalInput").ap()
    xti = nc.dram_tensor("xti", [128, 16 * B], _bf16, kind="ExternalInput").ap()
    xi = nc.dram_tensor("xi", [N_H, 128, P * B], _bf16, kind="ExternalInput").ap()
    # otd[jh][jm*16+d, jm2*B + b] = O[b, 8jh+jm2, d] if jm==jm2 else 0
    otd = nc.dram_tensor("otd", [JH, 128, 8 * B], _bf16,
                         kind="ExternalInput").ap()
    # sel[io*8+p, cc*128 + cc*16+io] = 1 (one selector block per cc)
    sel = nc.dram_tensor("sel", [128, 8 * 128], _bf16, kind="ExternalInput").ap()
    sp = nc.dram_tensor("sp", [B, JD], _f32, kind="ExternalOutput").ap()

    with ExitStack() as ctx:
        tc = ctx.enter_context(tile.TileContext(nc))
        inpool = ctx.enter_context(tc.tile_pool(name="in", bufs=1))
        apool = ctx.enter_context(tc.tile_pool(name="aps", bufs=2, space="PSUM"))
        gpool = ctx.enter_context(tc.tile_pool(name="gps", bufs=2, space="PSUM"))
        spool = ctx.enter_context(tc.tile_pool(name="sps", bufs=1, space="PSUM"))
        eapool = ctx.enter_context(tc.tile_pool(name="ea", bufs=6))
        mpool = ctx.enter_context(tc.tile_pool(name="m", bufs=10))
        cpool = ctx.enter_context(tc.tile_pool(name="cx", bufs=6))
        bigpool = ctx.enter_context(tc.tile_pool(name="big", bufs=1))
        zppool = ctx.enter_context(tc.tile_pool(name="zp", bufs=14))

        # ---- input loads (order matters for the pipeline head)
        w2t_t = []
        otd_t = []
        for jh in range(JH):
            t = inpool.tile([128, 16 * 128], _bf16, name=f"w2t_t{jh}")
            w2t_t.append(t)
            t2 = inpool.tile([128, 8 * B], _bf16, name=f"otd_t{jh}")
            otd_t.append(t2)
        sel_t = inpool.tile([128, 8 * 128], _bf16)
        xti_t = inpool.tile([128, 16 * B], _bf16)
        with tc.high_priority():
            nc.scalar.dma_start(otd_t[0][:], otd[0])
            nc.sync.dma_start(w2t_t[0][:], w2t[0])
            nc.scalar.dma_start(xti_t[:], xti[:])
            nc.sync.dma_start(sel_t[:], sel[:])
        for jh in range(1, JH):
            nc.sync.dma_start(w2t_t[jh][:], w2t[jh])
            nc.scalar.dma_start(otd_t[jh][:], otd[jh])

        # PE warm-up: ramp the tensor engine to full clock while the first
        # DMAs land (p-state model needs ~3us of continuous activity).
        wup = inpool.tile([128, 128], _bf16)
        nc.vector.memset(wup[:], 0.0)
        wu_ps = spool.tile([B, 512], _f32, name="wu_ps")
        for _ in range(10):
            nc.tensor.matmul(wu_ps[:, :128], lhsT=wup[:], rhs=wup[:],
                             start=True, stop=True)
        xi_t = []
        for h in range(N_H):
            t = inpool.tile([128, P * B], _bf16, name=f"xi_t{h}")
            nc.scalar.dma_start(t[:], xi[h])
            xi_t.append(t)
        w4_t = []
        for h in range(N_H):
            t = inpool.tile([128, P * JD], _bf16, name=f"w4_t{h}")
            nc.scalar.dma_start(t[:], w4[h])
            w4_t.append(t)

        eT = [bigpool.tile([128, J * B], _bf16, name=f"eT{h}") for h in range(N_H)]
        zparts = {0: [], 1: []}
        xh = [bigpool.tile([128, P * B], _bf16, name=f"xh{h}") for h in range(N_H)]

        def phase1_j(h, j, path):
            """A = W.O ; m = A*x for one (h, j)."""
            jm, jh = j % 8, j // 8
            a_ps = apool.tile([128, 8 * B], _f32)
            for cc in range(8):
                c = h * 8 + cc
                nc.tensor.matmul(
                    a_ps[:, cc * B:(cc + 1) * B],
                    lhsT=w2t_t[jh][:, c * 128:(c + 1) * 128],
                    rhs=otd_t[jh][:, jm * B:(jm + 1) * B],
                    start=True, stop=True,
                )
            m = mpool.tile([128, 8 * B], _bf16)
            xsl = xti_t[:, h * 8 * B:(h + 1) * 8 * B]
            if path == "d":
                nc.vector.tensor_tensor(
                    m[:], a_ps[:], xsl, op=mybir.AluOpType.mult)
            else:
                ea = eapool.tile([128, 8 * B], _bf16)
                nc.scalar.copy(ea[:], a_ps[:])
                eng = nc.gpsimd if path == "q" else nc.vector
                eng.tensor_tensor(
                    m[:], ea[:], xsl, op=mybir.AluOpType.mult)
            return m

        def sel_reduce(g_ps, m, jj):
            # g[:, jj] = sum_cc SEL_cc^T m_cc  (accumulating, full M=128)
            for cc in range(8):
                nc.tensor.matmul(
                    g_ps[:, jj * B:(jj + 1) * B],
                    lhsT=sel_t[:, cc * 128:(cc + 1) * 128],
                    rhs=m[:, cc * B:(cc + 1) * B],
                    start=(cc == 0), stop=(cc == 7),
                )

        # Software-pipelined phase 1 over one i-chunk h: stage1/evac/mult
        # for j runs STAGGER j-slots ahead of the SEL reduce for j, so the
        # PE never blocks on the Act->DVE chain. interleave(j) lets the
        # caller splice phase-2 work of the other chunk into each slot.
        STAGGER = 2

        def phase1_chunk(h, mix, interleave=None):
            pending = []   # (m_tile, j)
            g_tiles = {}
            for j in range(J + STAGGER):
                if j < J:
                    m = phase1_j(h, j, mix[j % len(mix)])
                    pending.append((m, j))
                if j >= STAGGER:
                    m0, j0 = pending.pop(0)
                    if j0 % GSZ == 0:
                        g_tiles[j0 // GSZ] = gpool.tile(
                            [128, GSZ * B], _f32, name="g_ps")
                    sel_reduce(g_tiles[j0 // GSZ], m0, j0 % GSZ)
                    if j0 % GSZ == GSZ - 1:
                        jg = j0 // GSZ
                        esl = eT[h][:, jg * GSZ * B:(jg + 1) * GSZ * B]
                        nc.scalar.activation(
                            esl, g_tiles.pop(jg)[:],
                            mybir.ActivationFunctionType.Exp,
                        )

                if interleave is not None and j < J:
                    interleave(j)

        def znorm(h):
            """Z = sum_j e (bf16 add tree over j-blocks); xh = x * (1/Z)."""
            e = eT[h]
            t1 = bigpool.tile([128, 16 * B], _bf16, name=f"zt1_{h}")
            nc.vector.tensor_tensor(
                t1[:], e[:, :16 * B], e[:, 16 * B:], op=mybir.AluOpType.add)
            t2 = bigpool.tile([128, 8 * B], _bf16, name=f"zt2_{h}")
            nc.vector.tensor_tensor(
                t2[:], t1[:, :8 * B], t1[:, 8 * B:], op=mybir.AluOpType.add)
            t3 = bigpool.tile([128, 4 * B], _bf16, name=f"zt3_{h}")
            nc.vector.tensor_tensor(
                t3[:], t2[:, :4 * B], t2[:, 4 * B:], op=mybir.AluOpType.add)
            t4 = bigpool.tile([128, 2 * B], _bf16, name=f"zt4_{h}")
            nc.vector.tensor_tensor(
                t4[:], t3[:, :2 * B], t3[:, 2 * B:], op=mybir.AluOpType.add)
            z = bigpool.tile([128, B], _f32, name=f"z_{h}")
            nc.vector.tensor_tensor(
                z[:], t4[:, :B], t4[:, B:], op=mybir.AluOpType.add)
            zr = bigpool.tile([128, B], _f32, name=f"zr_{h}")
            nc.vector.reciprocal(zr[:], z[:])
            zrb = bigpool.tile([128, B], _bf16)
            nc.scalar.copy(zrb[:], zr[:])
            nc.vector.tensor_tensor(
                xh[h][:].rearrange("i (p b) -> i p b", p=P, b=B),
                xi_t[h][:].rearrange("i (p b) -> i p b", p=P, b=B),
                zrb[:].unsqueeze(1).broadcast_to([128, P, B]),
                op=mybir.AluOpType.mult,
            )

        def phase2_j(h, j, pool=False):
            """cx = e_j * xh for a PAIR (j, j+1) in one instr; s += cx^T W."""
            cx = cpool.tile([128, 2 * P * B], _bf16)
            eng = nc.gpsimd if pool else nc.vector
            eng.tensor_tensor(
                cx[:].rearrange("i (j p b) -> i j p b", j=2, p=P, b=B),
                eT[h][:, j * B:(j + 2) * B]
                    .rearrange("i (j b) -> i j b", j=2, b=B)
                    .unsqueeze(2).broadcast_to([128, 2, P, B]),
                xh[h][:].rearrange("i (p b) -> i p b", p=P, b=B)
                    .unsqueeze(1).broadcast_to([128, 2, P, B]),
                op=mybir.AluOpType.mult,
            )
            for jj in range(2):
                for p in range(P):
                    nc.tensor.matmul(
                        s_ps[:, (j + jj) * D:(j + jj + 1) * D],
                        lhsT=cx[:, (jj * P + p) * B:(jj * P + p + 1) * B],
                        rhs=w4_t[h][:, (p * J + j + jj) * D:
                                    (p * J + j + jj + 1) * D],
                        start=False,
                        stop=(h == N_H - 1 and p == P - 1),
                        skip_group_check=True,
                    )

        # ---- schedule: P1(h0) | P1(h1) interleaved with P2(h0) | P2(h1)
        phase1_chunk(0, MIX_A)
        znorm(0)
        phase1_chunk(1, MIX_B,
                     interleave=lambda j: (phase2_j(
                         0, j, pool=(CX_B[(j // 2) % len(CX_B)] == "p"))
                         if j % 2 == 0 else None))
        znorm(1)
        sout = bigpool.tile([B, JD], _f32)
        for j in range(0, J, 2):
            phase2_j(1, j, pool=(CX_C[(j // 2) % len(CX_C)] == "p"))
            if j % 8 == 6:
                # this 8-j block of s columns is final; ship it now
                q0, q1 = (j - 6) * D, (j + 2) * D
                nc.scalar.copy(sout[:, q0:q1], s_ps[:, q0:q1])
                nc.sync.dma_start(sp[:, q0:q1], sout[:, q0:q1])
    return nc


# ---------------------------------------------------------------------------
# Host glue
def _squash(s):
    v = s.reshape(B, J, D)
    s2 = np.sum(np.square(v), axis=-1, keepdims=True)
    scale = s2 / (1.0 + s2) / np.sqrt(s2 + EPS)
    return (scale * v).astype(np.float32)


_cache = {}


def _get_nc(name):
    if name not in _cache:
        _cache[name] = build_l1() if name == "l1" else build_l2()
    return _cache[name]


def _prep_inputs(x, W):
    """Per-core host-side re-layouts (bf16)."""
    sel = np.zeros((128, 8 * 128), _bf)
    for cc in range(8):
        for io in range(16):
            sel[io * 8:(io + 1) * 8, cc * 128 + cc * 16 + io] = 1.0
    per_core = []
    for c in range(N_CORES):
        sl = slice(c * I_LOC, (c + 1) * I_LOC)
        xc = x[:, sl, :]                                   # [B, 256, P]
        wc = W[:, sl, :, :]                                # [J, 256, D, P]
        # xi[h, il, p*B+b] = x[b, 128h+il, p]
        xi = np.ascontiguousarray(
            xc.transpose(1, 2, 0).reshape(N_H, 128, P * B)).astype(_bf)
        # xti[io*8+p, c*B+b] = x[b, 16c+io, p]
        xti = np.ascontiguousarray(
            xc.reshape(B, 16, 16, P).transpose(2, 3, 1, 0)
              .reshape(128, 16 * B)).astype(_bf)
        # w4[h, il, (p*J+j)*D+d] = W[j, 128h+il, d, p]
        w4 = np.ascontiguousarray(
            wc.transpose(1, 3, 0, 2).reshape(N_H, 128, P * JD)).astype(_bf)
        # w2t[jh, jm*16+d, c*128 + io*8+p] = W[8jh+jm, 16c+io, d, p]
        w2 = (wc.reshape(JH, 8, 16, 16, D, P)      # [jh, jm, c, io, d, p]
                .transpose(0, 1, 4, 2, 3, 5)       # [jh, jm, d, c, io, p]
                .reshape(JH, 128, 16 * 128).astype(_bf))
        per_core.append(
            {"xi": xi, "xti": xti, "w4": w4, "sel": sel,
             "w2t": np.ascontiguousarray(w2)})
    return per_core


def _ot_layout(O):
    """O [B,J,D] f32 -> otd[jh, jm*16+d, jm2*B+b] = O[b,8jh+jm,d] iff jm==jm2."""
    o4 = O.reshape(B, JH, 8, D).transpose(1, 2, 3, 0)   # [jh, jm, d, b]
    z = np.zeros((JH, 8, D, 8, B), np.float32)
    for jm in range(8):
        z[:, jm, :, jm, :] = o4[:, jm]
    return z.reshape(JH, 128, 8 * B).astype(_bf)


def _run(nc, in_maps, **kw):
    return run_bass_kernel_spmd(nc, in_maps, list(range(N_CORES)), **kw)


def kernel(x, W, _collect_times=None):
    x = np.asarray(x, dtype=np.float32)
    W = np.asarray(W, dtype=np.float32)
    pc = _prep_inputs(x, W)

    nc1 = _get_nc("l1")
    nc2 = _get_nc("l2")

    r1 = _run(nc1, [{"xi": p["xi"], "w4": p["w4"]} for p in pc])
    s0 = np.sum([r1.results[c]["sp"] for c in range(N_CORES)], axis=0,
                dtype=np.float64).astype(np.float32)
    s0 *= (1.0 / J)
    out0 = _squash(s0)

    ot1 = _ot_layout(out0)
    l2_maps = [
        {k: p[k] for k in ("xi", "xti", "w4", "sel", "w2t")}
        for p in pc
    ]
    r2 = _run(nc2, [{**m, "otd": ot1} for m in l2_maps])
    s1 = np.sum([r2.results[c]["sp"] for c in range(N_CORES)], axis=0,
                dtype=np.float64).astype(np.float32)
    out1 = _squash(s1)
    ocum = out0 + out1

    ot2 = _ot_layout(ocum)
    r3 = _run(nc2, [{**m, "otd": ot2} for m in l2_maps])
    s2 = np.sum([r3.results[c]["sp"] for c in range(N_CORES)], axis=0,
                dtype=np.float64).astype(np.float32)
    out2 = _squash(s2)

    if _collect_times is not None:
        for r in (r1, r2, r3):
            _collect_times.append(r.exec_time_ns)
    return out2
